# revision 1
# baseline (speedup 1.0000x reference)
"""Trainium2 Bass kernel for nn_ChamferNormalLoss (8-core data parallel).

Sharding: pure data parallel — one batch sample per NeuronCore; the host
averages the 8 per-core |dot| sums (the only cross-core reduction).

Per-sample pipeline on each core:
  1. Brute-force NN searches (gt: 2048x8192, pred: 2048x2688-padded) as
     TensorE matmuls with a K=4 contraction that fuses the bias:
     s = 2*q.r - |r|^2, so argmax(s) == argmin squared distance.  The
     transposed [4, N] operand layouts are built on-chip with PE
     transposes (contiguous DMA loads only; column order is a known
     permutation of vertex id, unpermuted after the search with cheap
     int ops).  ScalarE evacuates distance PSUM tiles to SBUF; VectorE
     computes the row max with one 2x-mode tensor_scalar accumulate and
     extracts the argmax with max_index (first-match = jnp tie rule).
  2. Area-weighted vertex normals WITHOUT scatter support: face corner
     vertices are fetched with per-partition-row indirect DMAs (the only
     gather form the SWDGE ucode implements: one dense [128,1] offset
     column per instruction), cross products on VectorE, and the
     scatter-add n[v] += fn is factorized via v = hi*128 + lo: for each
     (face-chunk, corner), one-hot(lo) [128f,128lo] (fp16, built on
     GPSIMD) becomes matmul weights and one-hot(hi)*fn [128f,3*64]
     (fp16, VectorE two-op tensor_scalar) the moving operand; a single
     PSUM tile accumulates G[lo, c, hi] over all 384 chunk-corner pairs.
     One-hot emission is interleaved with the search chunks so VectorE
     stays busy while ScalarE evacuates.
  3. Epilogue: indirect row-gathers of the nearest gt normal (from G in
     DRAM) and nearest pred vertex; |dot| via dot/(max(|e|,eps)*
     max(|n|,eps)) == the reference's normalize-then-dot; abs-sum reduce
     per partition; partition sum via a ones-matmul into PSUM.

Numerics: fp32 throughout the searches (float32r/bf16 were measured to
flip ~2.5% of nearest-neighbor indices on HW); one-hots/fn in fp16 with
fp32 PSUM accumulation.  End-to-end relative error vs the fp32 jax
reference is ~5e-6 on hardware.
"""

import os, sys

for _p in (
    "/opt/trn_rl_repo",
    "/opt/pypackages",
    "/root/.axon_site/_ro/trn_rl_repo",
    "/root/.axon_site/_ro/pypackages",
):
    if os.path.isdir(_p) and _p not in sys.path:
        sys.path.insert(0, _p)

import numpy as np

import concourse.bass as bass
import concourse.bacc as bacc
import concourse.tile as tile
from concourse import masks, mybir

F32 = mybir.dt.float32
FP16 = mybir.dt.float16
I32 = mybir.dt.int32
U32 = mybir.dt.uint32
A = mybir.AluOpType
AF = mybir.ActivationFunctionType
AX = mybir.AxisListType

B = 8
P, PC = 2048, 16          # queries, chunks of 128
NGT, CGT = 8192, 16       # gt vertices, n-chunks of 512
VPR, VPAD, CPR = 2562, 2688, 6
NF, FCH = 16384, 128      # faces, chunks of 128
BIGC = 1.0e6              # pad coordinate; rsq pad = 3e12

EPS = 1e-12


def build_nc(debug_outs=False):
    nc = bacc.Bacc(None, target_bir_lowering=False)
    pp = nc.dram_tensor("pred_points", [P, 3], F32, kind="ExternalInput")
    pv = nc.dram_tensor("pred_vertices", [VPR, 3], F32, kind="ExternalInput")
    gv = nc.dram_tensor("gt_vertices", [NGT, 3], F32, kind="ExternalInput")
    gf = nc.dram_tensor("gt_faces32", [NF, 3], I32, kind="ExternalInput")
    g_dram = nc.dram_tensor("g_norm", [NGT * 3, 1], F32)
    out = nc.dram_tensor("loss_sum", [1], F32, kind="ExternalOutput")

    from contextlib import ExitStack

    dbg = {}
    if debug_outs:
        for nm, shape, dt in [
            ("dbg_idx_gt", [128, PC], I32), ("dbg_idx_pr", [128, PC], I32),
            ("dbg_dot", [128, PC], F32), ("dbg_ee", [128, PC], F32),
            ("dbg_nn", [128, PC], F32), ("dbg_res", [128, PC], F32),
            ("dbg_g", [128, 192], F32), ("dbg_fn", [128, FCH * 3], F32),
            ("dbg_rt", [4, NGT], F32), ("dbg_rtp", [4, VPAD], F32),
            ("dbg_qt", [4, P], F32),
        ]:
            dbg[nm] = nc.dram_tensor(nm, shape, dt, kind="ExternalOutput")
    with tile.TileContext(nc) as tc, ExitStack() as ctx:
        _body(nc, tc, ctx, pp, pv, gv, gf, g_dram, out, dbg)
    nc.compile()
    return nc


def _body(nc, tc, ctx, pp, pv, gv, gf, g_dram, out_dram, dbg=None):
    sing = ctx.enter_context(tc.tile_pool(name="sing", bufs=1))
    work = ctx.enter_context(tc.tile_pool(name="work", bufs=2))
    oh = ctx.enter_context(tc.tile_pool(name="oh", bufs=3))
    ppsum = ctx.enter_context(
        tc.tile_pool(name="ppsum", bufs=4, space=bass.MemorySpace.PSUM)
    )
    mpsum = ctx.enter_context(
        tc.tile_pool(name="mpsum", bufs=1, space=bass.MemorySpace.PSUM)
    )
    gpsum = ctx.enter_context(
        tc.tile_pool(name="gpsum", bufs=1, space=bass.MemorySpace.PSUM)
    )

    ident0 = sing.tile([128, 128], F32)
    masks.make_identity(nc, ident0[:])
    # transpose-mode matmuls can carry only one sync wait, so make the
    # identity a DVE product: every transpose then waits on DVE alone.
    ident = sing.tile([128, 128], F32)
    nc.vector.tensor_copy(ident[:], ident0[:])

    # ---- query side: qT[:, n] = [2qx, 2qy, 2qz, -1] of query (n&127)*16 + (n>>7)
    qRM = sing.tile([128, PC, 3], F32)
    nc.sync.dma_start(out=qRM[:], in_=pp[:, :].rearrange("(p i) c -> p i c", p=128))
    qCM = work.tile([128, 3, PC], F32, tag="qcm")
    nc.vector.tensor_copy(qCM[:], qRM[:].rearrange("p i c -> p c i"))
    qT = sing.tile([4, P], F32)
    nc.vector.memset(qT[:, :], -1.0)
    qtp = mpsum.tile([48, 128], F32, tag="tp")
    nc.tensor.transpose(qtp[:], qCM[:].rearrange("p c i -> p (c i)"), ident[:])
    qtsb = work.tile([48, 128], F32, tag="tsb")
    nc.vector.tensor_scalar(
        out=qtsb[:], in0=qtp[:], scalar1=2.0, scalar2=None, op0=A.mult
    )
    nc.sync.dma_start(
        out=qT[0:3, :].rearrange("c (i p) -> c i p", p=128), in_=qtsb[:]
    )

    # ---- gt side: rT[:, n] = [x, y, z, |r|^2] of vertex (n&127)*64 + (n>>7)
    rRM = work.tile([128, 64, 3], F32, tag="rrm")
    nc.sync.dma_start(out=rRM[:], in_=gv[:, :].rearrange("(p t) c -> p t c", p=128))
    rCM = work.tile([128, 2, 3, 32], F32, tag="rcm")
    nc.vector.tensor_copy(rCM[:], rRM[:].rearrange("p (h t) c -> p h c t", h=2))
    sq = work.tile([128, 64, 3], F32, tag="sq")
    nc.vector.tensor_tensor(out=sq[:], in0=rRM[:], in1=rRM[:], op=A.mult)
    rsq = work.tile([128, 64], F32, tag="rsq")
    nc.vector.tensor_reduce(out=rsq[:], in_=sq[:], axis=AX.X, op=A.add)
    rT = sing.tile([4, NGT], F32)
    for h in range(2):
        ctp = mpsum.tile([96, 128], F32, tag="tp")
        nc.tensor.transpose(
            ctp[:], rCM[:, h, :, :].rearrange("p c t -> p (c t)"), ident[:]
        )
        ctsb = work.tile([96, 128], F32, tag="tsb")
        nc.vector.tensor_copy(ctsb[:], ctp[:])
        nc.sync.dma_start(
            out=rT[0:3, h * 32 * 128 : (h + 1) * 32 * 128].rearrange(
                "c (t p) -> c t p", p=128
            ),
            in_=ctsb[:],
        )
    stp = mpsum.tile([64, 128], F32, tag="tp")
    nc.tensor.transpose(stp[:], rsq[:], ident[:])
    stsb = work.tile([64, 128], F32, tag="tsb")
    nc.vector.tensor_copy(stsb[:], stp[:])
    nc.sync.dma_start(out=rT[3:4, :], in_=stsb[:])

    # ---- pred side (padded to 2688): vertex (n&127)*21 + (n>>7)
    rRMp = work.tile([128, 21, 3], F32, tag="rrmp")
    nc.vector.memset(rRMp[:], BIGC)
    rRMp_f = rRMp[:].rearrange("p t c -> p (t c)")
    pv_f = pv[:, :].rearrange("v c -> (v c)")
    nc.sync.dma_start(
        out=rRMp_f[0:122, :],
        in_=pv_f[0 : 122 * 63].rearrange("(p a) -> p a", a=63),
    )
    rCMp = work.tile([128, 3, 21], F32, tag="rcmp")
    nc.vector.tensor_copy(rCMp[:], rRMp[:].rearrange("p t c -> p c t"))
    sqp = work.tile([128, 21, 3], F32, tag="sqp")
    nc.vector.tensor_tensor(out=sqp[:], in0=rRMp[:], in1=rRMp[:], op=A.mult)
    rsqp = work.tile([128, 21], F32, tag="rsqp")
    nc.vector.tensor_reduce(out=rsqp[:], in_=sqp[:], axis=AX.X, op=A.add)
    rTp = sing.tile([4, VPAD], F32)
    ptp = mpsum.tile([63, 128], F32, tag="tp")
    nc.tensor.transpose(ptp[:], rCMp[:].rearrange("p c t -> p (c t)"), ident[:])
    ptsb = work.tile([63, 128], F32, tag="tsb")
    nc.vector.tensor_copy(ptsb[:], ptp[:])
    nc.sync.dma_start(
        out=rTp[0:3, :].rearrange("c (t p) -> c t p", p=128), in_=ptsb[:]
    )
    sptp = mpsum.tile([21, 128], F32, tag="tp")
    nc.tensor.transpose(sptp[:], rsqp[:], ident[:])
    sptsb = work.tile([21, 128], F32, tag="tsb")
    nc.vector.tensor_copy(sptsb[:], sptp[:])
    nc.sync.dma_start(out=rTp[3:4, :], in_=sptsb[:])

    # ---------------- faces: corner indices, lo/hi decomposition ----------
    faces = sing.tile([128, FCH, 3], I32)
    nc.sync.dma_start(
        out=faces[:], in_=gf[:, :].rearrange("(p ch) w -> p ch w", p=128)
    )
    lo_i = sing.tile([128, FCH, 3], I32)
    hi_i = sing.tile([128, FCH, 3], I32)
    nc.vector.tensor_scalar(
        out=lo_i[:], in0=faces[:], scalar1=127, scalar2=None, op0=A.bitwise_and
    )
    nc.vector.tensor_scalar(
        out=hi_i[:], in0=faces[:], scalar1=7, scalar2=None, op0=A.logical_shift_right
    )
    lo_f = sing.tile([128, FCH, 3], F32)
    hi_f = sing.tile([128, FCH, 3], F32)
    nc.vector.tensor_copy(lo_f[:], lo_i[:])
    nc.vector.tensor_copy(hi_f[:], hi_i[:])

    # ---------------- iotas ----------------
    io128_i = sing.tile([128, 128], I32)
    nc.gpsimd.iota(io128_i[:], pattern=[[1, 128]], base=0, channel_multiplier=0)
    io128 = sing.tile([128, 128], FP16)
    nc.vector.tensor_copy(io128[:], io128_i[:])
    io64_i = sing.tile([128, 64], I32)
    nc.gpsimd.iota(io64_i[:], pattern=[[1, 64]], base=0, channel_multiplier=0)
    io64 = sing.tile([128, 64], FP16)
    nc.vector.tensor_copy(io64[:], io64_i[:])

    # ---------------- gather face corner vertices, cross products ---------
    faces3 = sing.tile([128, FCH, 3], I32)
    nc.vector.tensor_scalar(
        out=faces3[:], in0=faces[:], scalar1=3, scalar2=None, op0=A.mult
    )
    gv_flat = gv[:, :].rearrange("v (c one) -> (v c) one", one=1)
    Vg = sing.tile([128, FCH * 3, 3], F32)
    gcols = ctx.enter_context(tc.tile_pool(name="gcols", bufs=8))
    for j in range(FCH * 3):
        col = gcols.tile([128, 1], I32, tag="gcol")
        nc.scalar.copy(col[:], faces3[:].rearrange("p a b -> p (a b)")[:, j : j + 1])
        nc.gpsimd.indirect_dma_start(
            out=Vg[:, j, :],
            out_offset=None,
            in_=gv_flat,
            in_offset=bass.IndirectOffsetOnAxis(ap=col[:], axis=0),
        )
    Vg4 = Vg[:].rearrange("p (ch c) d -> p ch c d", c=3)
    eA = sing.tile([128, FCH, 3], F32)
    eB = sing.tile([128, FCH, 3], F32)
    nc.vector.tensor_tensor(
        out=eA[:], in0=Vg4[:, :, 1, :], in1=Vg4[:, :, 0, :], op=A.subtract
    )
    nc.vector.tensor_tensor(
        out=eB[:], in0=Vg4[:, :, 2, :], in1=Vg4[:, :, 0, :], op=A.subtract
    )
    fn = sing.tile([128, FCH, 3], F32)
    for d in range(3):
        u, v = (d + 1) % 3, (d + 2) % 3
        t1 = work.tile([128, FCH], F32, tag="cr1")
        t2 = work.tile([128, FCH], F32, tag="cr2")
        nc.vector.tensor_tensor(out=t1[:], in0=eA[:, :, u], in1=eB[:, :, v], op=A.mult)
        nc.vector.tensor_tensor(out=t2[:], in0=eA[:, :, v], in1=eB[:, :, u], op=A.mult)
        nc.vector.tensor_tensor(out=fn[:, :, d], in0=t1[:], in1=t2[:], op=A.subtract)

    # ---------------- NN searches ----------------
    idx_gt = sing.tile([128, PC], I32)
    idx_pr = sing.tile([128, PC], I32)

    F32R = mybir.dt.float32r

    NEG = -3.0e38

    def search_chunk(rT_t, ncols, nch, idx_out, i):
        s_sb = work.tile([128, ncols], F32, tag="s")
        for c in range(nch):
            n0 = c * 512
            n1 = min(n0 + 512, ncols)
            w = n1 - n0
            ps = ppsum.tile([128, 512], F32, tag="d")
            nc.tensor.matmul(
                ps[:, 0:w],
                qT[:, i * 128 : (i + 1) * 128],
                rT_t[:, n0:n1],
                start=True,
                stop=True,
            )
            nc.scalar.copy(s_sb[:, n0:n1], ps[:, 0:w])
        # full-row max at 2x mode (fp32 SBUF single-src), in-place identity
        rmax = work.tile([128, 1], F32, tag="rmax")
        nc.vector.tensor_scalar(
            out=s_sb[:], in0=s_sb[:], scalar1=NEG, scalar2=None,
            op0=A.max, op1=A.max, accum_out=rmax[:],
        )
        mx8 = work.tile([128, 8], F32, tag="mx8")
        nc.vector.tensor_copy(mx8[:], rmax[:].to_broadcast([128, 8]))
        ix8 = work.tile([128, 8], U32, tag="ix8")
        nc.vector.max_index(ix8[:], mx8[:], s_sb[:])
        nc.vector.tensor_copy(idx_out[:, i : i + 1], ix8[:, 0:1])

    # ---------------- one-hot scatter: G[lo, c, hi] ----------------
    # emission interleaved with the NN-search chunks: the one-hot builds keep
    # the DVE busy while ScalarE evacuates search PSUM tiles.
    Gp = gpsum.tile([128, 3, 64], F32)
    _oh_state = {"k": 0}

    def emit_onehot(n):
        for _ in range(n):
            k = _oh_state["k"]
            if k >= 3 * FCH:
                return
            ch, corner = divmod(k, 3)
            ohlo = oh.tile([128, 128], FP16, tag="ohlo")
            nc.gpsimd.tensor_scalar(
                out=ohlo[:],
                in0=io128[:],
                scalar1=lo_f[:, ch : ch + 1, corner : corner + 1],
                scalar2=None,
                op0=A.is_equal,
            )
            R = oh.tile([128, 3, 64], FP16, tag="R")
            for d in range(3):
                nc.vector.tensor_scalar(
                    out=R[:, d, :],
                    in0=io64[:],
                    scalar1=hi_f[:, ch : ch + 1, corner : corner + 1],
                    scalar2=fn[:, ch : ch + 1, d : d + 1],
                    op0=A.is_equal,
                    op1=A.mult,
                )
            nc.tensor.matmul(
                Gp[:],
                ohlo[:],
                R[:],
                start=(k == 0),
                stop=(k == 3 * FCH - 1),
                skip_group_check=True,
            )
            _oh_state["k"] = k + 1

    for i in range(PC):
        search_chunk(rT, NGT, CGT, idx_gt, i)
        emit_onehot(24)
    emit_onehot(3 * FCH)  # leftovers

    # ---- unpermute column index n -> vertex id ----
    def unpermute(idx_t, mult):
        a = sing.tile([128, PC], I32, tag=f"unp_a{mult}")
        bcol = sing.tile([128, PC], I32, tag=f"unp_b{mult}")
        nc.vector.tensor_scalar(
            out=a[:], in0=idx_t[:], scalar1=127, scalar2=None, op0=A.bitwise_and
        )
        nc.vector.tensor_scalar(
            out=a[:], in0=a[:], scalar1=mult, scalar2=None, op0=A.mult
        )
        nc.vector.tensor_scalar(
            out=bcol[:], in0=idx_t[:], scalar1=7, scalar2=None, op0=A.logical_shift_right
        )
        nc.vector.tensor_tensor(out=idx_t[:], in0=a[:], in1=bcol[:], op=A.add)

    unpermute(idx_gt, 64)

    Gs = sing.tile([128, 3, 64], F32)
    nc.scalar.copy(Gs[:], Gp[:])
    Gs2 = sing.tile([128, 64, 3], F32)
    nc.vector.tensor_copy(Gs2[:], Gs[:].rearrange("p c h -> p h c"))
    nc.sync.dma_start(
        out=g_dram[:, :].rearrange("(lo hi c) one -> lo (hi c one)", lo=128, hi=64),
        in_=Gs2[:],
    )

    # gather offsets for normals: (v & 127)*192 + (v >> 7)*3
    o1 = sing.tile([128, PC], I32)
    o2 = sing.tile([128, PC], I32)
    nc.vector.tensor_scalar(
        out=o1[:], in0=idx_gt[:], scalar1=127, scalar2=None, op0=A.bitwise_and
    )
    nc.vector.tensor_scalar(
        out=o1[:], in0=o1[:], scalar1=192, scalar2=None, op0=A.mult
    )
    nc.vector.tensor_scalar(
        out=o2[:], in0=idx_gt[:], scalar1=7, scalar2=None, op0=A.logical_shift_right
    )
    nc.vector.tensor_scalar(
        out=o2[:], in0=o2[:], scalar1=3, scalar2=None, op0=A.mult
    )
    offs = sing.tile([128, PC], I32)
    nc.vector.tensor_tensor(out=offs[:], in0=o1[:], in1=o2[:], op=A.add)

    nGT = sing.tile([128, PC, 3], F32)
    for i in range(PC):
        col = gcols.tile([128, 1], I32, tag="gcol")
        nc.scalar.copy(col[:], offs[:, i : i + 1])
        nc.gpsimd.indirect_dma_start(
            out=nGT[:, i, :],
            out_offset=None,
            in_=g_dram[:, :],
            in_offset=bass.IndirectOffsetOnAxis(ap=col[:], axis=0),
        )

    for i in range(PC):
        search_chunk(rTp, VPAD, CPR, idx_pr, i)
    unpermute(idx_pr, 21)



    # ---------------- epilogue ----------------
    idx_pr3 = sing.tile([128, PC], I32)
    nc.vector.tensor_scalar(
        out=idx_pr3[:], in0=idx_pr[:], scalar1=3, scalar2=None, op0=A.mult
    )
    pv_flat2 = pv[:, :].rearrange("v (c one) -> (v c) one", one=1)
    vNN = sing.tile([128, PC, 3], F32)
    for i in range(PC):
        col = gcols.tile([128, 1], I32, tag="gcol")
        nc.scalar.copy(col[:], idx_pr3[:, i : i + 1])
        nc.gpsimd.indirect_dma_start(
            out=vNN[:, i, :],
            out_offset=None,
            in_=pv_flat2,
            in_offset=bass.IndirectOffsetOnAxis(ap=col[:], axis=0),
        )
    e = sing.tile([128, PC, 3], F32)
    nc.vector.tensor_tensor(out=e[:], in0=qRM[:], in1=vNN[:], op=A.subtract)
    tmp3 = work.tile([128, PC, 3], F32, tag="en")
    nc.vector.tensor_tensor(out=tmp3[:], in0=e[:], in1=nGT[:], op=A.mult)
    dot = sing.tile([128, PC], F32)
    nc.vector.tensor_reduce(out=dot[:], in_=tmp3[:], axis=AX.X, op=A.add)
    ee_t = work.tile([128, PC, 3], F32, tag="en")
    nc.vector.tensor_tensor(out=ee_t[:], in0=e[:], in1=e[:], op=A.mult)
    ee = sing.tile([128, PC], F32)
    nc.vector.tensor_reduce(out=ee[:], in_=ee_t[:], axis=AX.X, op=A.add)
    nn_t = work.tile([128, PC, 3], F32, tag="en")
    nc.vector.tensor_tensor(out=nn_t[:], in0=nGT[:], in1=nGT[:], op=A.mult)
    nn = sing.tile([128, PC], F32)
    nc.vector.tensor_reduce(out=nn[:], in_=nn_t[:], axis=AX.X, op=A.add)

    elen = sing.tile([128, PC], F32)
    nlen = sing.tile([128, PC], F32)
    nc.scalar.activation(elen[:], ee[:], AF.Sqrt)
    nc.scalar.activation(nlen[:], nn[:], AF.Sqrt)
    nc.vector.tensor_scalar(
        out=elen[:], in0=elen[:], scalar1=EPS, scalar2=None, op0=A.max
    )
    nc.vector.tensor_scalar(
        out=nlen[:], in0=nlen[:], scalar1=EPS, scalar2=None, op0=A.max
    )
    den = sing.tile([128, PC], F32)
    nc.vector.tensor_tensor(out=den[:], in0=elen[:], in1=nlen[:], op=A.mult)
    rden = sing.tile([128, PC], F32)
    nc.vector.reciprocal(rden[:], den[:])
    res = sing.tile([128, PC], F32)
    nc.vector.tensor_tensor(out=res[:], in0=dot[:], in1=rden[:], op=A.mult)
    partial = sing.tile([128, 1], F32)
    nc.vector.tensor_reduce(
        out=partial[:], in_=res[:], axis=AX.X, op=A.add, apply_absolute_value=True
    )
    ones = sing.tile([128, 1], F32)
    nc.vector.memset(ones[:], 1.0)
    fps = mpsum.tile([1, 1], F32, tag="fin")
    nc.tensor.matmul(fps[:], ones[:], partial[:], start=True, stop=True)
    osb = sing.tile([1, 1], F32)
    nc.scalar.copy(osb[:], fps[:])
    nc.sync.dma_start(out=out_dram[:], in_=osb[:])
    if dbg:
        nc.sync.dma_start(out=dbg["dbg_idx_gt"][:, :], in_=idx_gt[:])
        nc.sync.dma_start(out=dbg["dbg_idx_pr"][:, :], in_=idx_pr[:])
        nc.sync.dma_start(out=dbg["dbg_dot"][:, :], in_=dot[:])
        nc.sync.dma_start(out=dbg["dbg_ee"][:, :], in_=ee[:])
        nc.sync.dma_start(out=dbg["dbg_nn"][:, :], in_=nn[:])
        nc.sync.dma_start(out=dbg["dbg_res"][:, :], in_=res[:])
        nc.sync.dma_start(out=dbg["dbg_g"][:, :], in_=Gs2[:].rearrange("p a b -> p (a b)"))
        nc.sync.dma_start(out=dbg["dbg_fn"][:, :], in_=fn[:].rearrange("p a b -> p (a b)"))
        nc.sync.dma_start(out=dbg["dbg_rt"][:, :], in_=rT[:])
        nc.sync.dma_start(out=dbg["dbg_rtp"][:, :], in_=rTp[:])
        nc.sync.dma_start(out=dbg["dbg_qt"][:, :], in_=qT[:])


_NC_CACHE = None


def _get_nc():
    global _NC_CACHE
    if _NC_CACHE is None:
        _NC_CACHE = build_nc()
    return _NC_CACHE


def make_in_maps(pred_points, pred_vertices, gt_vertices, gt_faces):
    nb = pred_points.shape[0]
    faces32 = np.asarray(gt_faces).astype(np.int32, copy=False)
    return [
        dict(
            pred_points=np.ascontiguousarray(pred_points[b], dtype=np.float32),
            pred_vertices=np.ascontiguousarray(pred_vertices[b], dtype=np.float32),
            gt_vertices=np.ascontiguousarray(gt_vertices[b], dtype=np.float32),
            gt_faces32=np.ascontiguousarray(faces32[b]),
        )
        for b in range(nb)
    ]


def kernel(pred_points, pred_vertices, gt_vertices, gt_faces):
    from concourse.bass_utils import run_bass_kernel_spmd

    nb = pred_points.shape[0]
    nc = _get_nc()
    in_maps = make_in_maps(pred_points, pred_vertices, gt_vertices, gt_faces)
    res = run_bass_kernel_spmd(nc, in_maps, list(range(nb)))
    total = sum(float(res.results[i]["loss_sum"][0]) for i in range(nb))
    return np.array(total / (nb * P), dtype=np.float32)


if __name__ == "__main__":
    nc = build_nc()
    print("built ok")



# revision 4
# speedup vs baseline: 1.0541x; 1.0541x over previous
"""Trainium2 Bass kernel for nn_ChamferNormalLoss (8-core data parallel).

Sharding: pure data parallel - one batch sample per NeuronCore; the host
averages the 8 per-core |dot| sums (the only cross-core reduction).

Per-sample pipeline on each core (v2):
  1. NN searches as TensorE float32r matmuls (1 cyc/row vs fp32's 4) with a
     K=4 contraction fusing the bias: s = 2*q.r - |r|^2.  Reduction via a
     windowed fp16 pack: a strided GPSIMD sample pass over PSUM gives a
     row-max estimate m^; ScalarE evacuates PSUM with
     relu((s - m^ + 1)*2^8 + 1024) to fp16, which lands winners in
     [1024,2048) where fp16 ulp=1 rounds scores to integers for free; one
     VectorE tensor_tensor adds iota*2^-13 (span < 1 = the fp16 ulp, so
     score order can never be violated) into an fp32 pk and a 2x max-accum
     pass yields pk_max = q* + j*2^-13; j* decodes with integer ops.
     Quantization (2^-8 in score units) flips only near-tie neighbours,
     which leaves the mean |dot| loss unchanged to ~1e-3 (measured).
  2. Face corner coordinates are host-pregathered (gv[faces] is pure input
     prep); cross products on VectorE; the scatter-add n[v] += fn is
     factorized via v = hi*128 + lo: per (face-chunk, corner) a one-hot(lo)
     [128f,128lo] fp16 (VectorE) is the matmul weight and one-hot(hi)*fn
     [128f,3*64] fp16 (GPSIMD) the moving operand; one PSUM tile
     accumulates G[lo,c,hi] over all 384 chunk-corner pairs, interleaved
     with the search chunks to keep all engines busy.
  3. Epilogue: batched multi-column indirect row-gathers of nearest gt
     normals (from G in DRAM) and nearest pred vertices; |dot| via
     dot/(max(|e|,eps)*max(|n|,eps)); abs-sum reduce; partition sum via a
     ones-matmul.
"""

import os, sys

for _p in (
    "/opt/trn_rl_repo",
    "/opt/pypackages",
    "/root/.axon_site/_ro/trn_rl_repo",
    "/root/.axon_site/_ro/pypackages",
):
    if os.path.isdir(_p) and _p not in sys.path:
        sys.path.insert(0, _p)

import numpy as np

import concourse.bass as bass
import concourse.bacc as bacc
import concourse.tile as tile
from concourse import masks, mybir

F32 = mybir.dt.float32
F16 = mybir.dt.float16
F32R = mybir.dt.float32r
I32 = mybir.dt.int32
A = mybir.AluOpType
AF = mybir.ActivationFunctionType
AX = mybir.AxisListType

B = 8
P, PC = 2048, 16          # queries, chunks of 128
NGT = 8192                # gt vertices; searched as 16 chunks of 512
VPR, VPAD = 2562, 2688    # pred vertices, padded to 21*128
NF, FCH = 16384, 128      # faces, chunks of 128
BIGC = 1.0e6              # pad coordinate; score approx -3e12

EPS = 1e-12
NEG = -3.0e38
WSC = 256.0               # 2^8 window scale
IOS = 2.0 ** -13          # iota step

# gt groups: 16 chunks of 512 -> PSUM groups of (3,3,3,3,3,1) chunks
GT_GROUPS = [(0, 3), (3, 6), (6, 9), (9, 12), (12, 15), (15, 16)]
# pred: 2688 cols = chunks (512*5 + 128) -> groups of (3 chunks, 2+tail)
PR_GROUPS = [(0, 1536), (1536, 2688)]


def build_nc(debug_outs=False):
    nc = bacc.Bacc(None, target_bir_lowering=False)
    pp = nc.dram_tensor("pred_points", [P, 3], F32, kind="ExternalInput")
    pv = nc.dram_tensor("pred_vertices", [VPR, 3], F32, kind="ExternalInput")
    gv = nc.dram_tensor("gt_vertices", [NGT, 3], F32, kind="ExternalInput")
    gf = nc.dram_tensor("gt_faces32", [NF, 3], I32, kind="ExternalInput")
    crn = nc.dram_tensor("corners", [NF, 3, 3], F32, kind="ExternalInput")
    g_dram = nc.dram_tensor("g_norm", [NGT * 3, 1], F32)
    out = nc.dram_tensor("loss_sum", [1], F32, kind="ExternalOutput")

    from contextlib import ExitStack

    dbg = {}
    if debug_outs:
        for nm, shape, dt in [
            ("dbg_idx_gt", [128, PC], I32), ("dbg_idx_pr", [128, PC], I32),
            ("dbg_pkg", [128, PC], F32), ("dbg_pkp", [128, PC], F32),
            ("dbg_res", [128, PC], F32), ("dbg_g", [128, 192], F32),
            ("dbg_evg", [128, NGT], F16), ("dbg_fn", [128, FCH * 3], F32),
        ]:
            dbg[nm] = nc.dram_tensor(nm, shape, dt, kind="ExternalOutput")
    with tile.TileContext(nc) as tc, ExitStack() as ctx:
        _body(nc, tc, ctx, pp, pv, gv, gf, crn, g_dram, out, dbg)
    nc.compile()
    return nc


def _body(nc, tc, ctx, pp, pv, gv, gf, crn, g_dram, out_dram, dbg=None):
    sing = ctx.enter_context(tc.tile_pool(name="sing", bufs=1))
    work = ctx.enter_context(tc.tile_pool(name="work", bufs=2))
    oh = ctx.enter_context(tc.tile_pool(name="oh", bufs=3))
    evp = ctx.enter_context(tc.tile_pool(name="evp", bufs=2))
    pkp = ctx.enter_context(tc.tile_pool(name="pkp", bufs=1))
    spsum = ctx.enter_context(
        tc.tile_pool(name="spsum", bufs=2, space=bass.MemorySpace.PSUM)
    )
    mpsum = ctx.enter_context(
        tc.tile_pool(name="mpsum", bufs=1, space=bass.MemorySpace.PSUM)
    )
    gpsum = ctx.enter_context(
        tc.tile_pool(name="gpsum", bufs=1, space=bass.MemorySpace.PSUM)
    )

    ident0 = sing.tile([128, 128], F32)
    masks.make_identity(nc, ident0[:])
    ident = sing.tile([128, 128], F32)
    nc.vector.tensor_copy(ident[:], ident0[:])

    # ---- corners: [16384,3,3] -> [128, FCH, 3, 3], face f = p*FCH + ch
    crnT = sing.tile([128, FCH, 3, 3], F32)
    crn_r = crn[:, :, :].rearrange("(p ch) c d -> p ch c d", p=128)
    for part in range(8):
        nc.sync.dma_start(
            out=crnT[:, part * 16:(part + 1) * 16, :, :],
            in_=crn_r[:, part * 16:(part + 1) * 16, :, :],
        )

    # ---- query side: qT[:, n] = [2qx,2qy,2qz,-1], query (n&127)*16 + (n>>7)
    qRM = sing.tile([128, PC, 3], F32)
    nc.sync.dma_start(out=qRM[:], in_=pp[:, :].rearrange("(p i) c -> p i c", p=128))
    qCM = work.tile([128, 3, PC], F32, tag="qcm")
    nc.vector.tensor_copy(qCM[:], qRM[:].rearrange("p i c -> p c i"))
    qT = sing.tile([4, P], F32)
    nc.vector.memset(qT[:, :], -1.0)
    qtp = mpsum.tile([48, 128], F32, tag="tp")
    nc.tensor.transpose(qtp[:], qCM[:].rearrange("p c i -> p (c i)"), ident[:])
    qtsb = work.tile([48, 128], F32, tag="tsb")
    nc.vector.tensor_scalar(
        out=qtsb[:], in0=qtp[:], scalar1=2.0, scalar2=None, op0=A.mult
    )
    nc.sync.dma_start(
        out=qT[0:3, :].rearrange("c (i p) -> c i p", p=128), in_=qtsb[:]
    )

    # ---- gt side: rT[:, n] = [x,y,z,|r|^2], vertex (n&127)*64 + (n>>7)
    rRM = work.tile([128, 64, 3], F32, tag="rrm")
    nc.sync.dma_start(out=rRM[:], in_=gv[:, :].rearrange("(p t) c -> p t c", p=128))
    rCM = work.tile([128, 2, 3, 32], F32, tag="rcm")
    nc.vector.tensor_copy(rCM[:], rRM[:].rearrange("p (h t) c -> p h c t", h=2))
    sq = work.tile([128, 64, 3], F32, tag="sq")
    nc.vector.tensor_tensor(out=sq[:], in0=rRM[:], in1=rRM[:], op=A.mult)
    rsq = work.tile([128, 64], F32, tag="rsq")
    nc.vector.tensor_reduce(out=rsq[:], in_=sq[:], axis=AX.X, op=A.add)
    rT = sing.tile([4, NGT], F32)
    for h in range(2):
        ctp = mpsum.tile([96, 128], F32, tag="tp")
        nc.tensor.transpose(
            ctp[:], rCM[:, h, :, :].rearrange("p c t -> p (c t)"), ident[:]
        )
        ctsb = work.tile([96, 128], F32, tag="tsb")
        nc.vector.tensor_copy(ctsb[:], ctp[:])
        nc.sync.dma_start(
            out=rT[0:3, h * 32 * 128 : (h + 1) * 32 * 128].rearrange(
                "c (t p) -> c t p", p=128
            ),
            in_=ctsb[:],
        )
    stp = mpsum.tile([64, 128], F32, tag="tp")
    nc.tensor.transpose(stp[:], rsq[:], ident[:])
    stsb = work.tile([64, 128], F32, tag="tsb")
    nc.vector.tensor_copy(stsb[:], stp[:])
    nc.sync.dma_start(out=rT[3:4, :], in_=stsb[:])

    # ---- pred side (padded to 2688): vertex (n&127)*21 + (n>>7)
    rRMp = work.tile([128, 21, 3], F32, tag="rrmp")
    nc.vector.memset(rRMp[:], BIGC)
    rRMp_f = rRMp[:].rearrange("p t c -> p (t c)")
    pv_f = pv[:, :].rearrange("v c -> (v c)")
    nc.sync.dma_start(
        out=rRMp_f[0:122, :],
        in_=pv_f[0 : 122 * 63].rearrange("(p a) -> p a", a=63),
    )
    rCMp = work.tile([128, 3, 21], F32, tag="rcmp")
    nc.vector.tensor_copy(rCMp[:], rRMp[:].rearrange("p t c -> p c t"))
    sqp = work.tile([128, 21, 3], F32, tag="sqp")
    nc.vector.tensor_tensor(out=sqp[:], in0=rRMp[:], in1=rRMp[:], op=A.mult)
    rsqp = work.tile([128, 21], F32, tag="rsqp")
    nc.vector.tensor_reduce(out=rsqp[:], in_=sqp[:], axis=AX.X, op=A.add)
    rTp = sing.tile([4, VPAD], F32)
    ptp = mpsum.tile([63, 128], F32, tag="tp")
    nc.tensor.transpose(ptp[:], rCMp[:].rearrange("p c t -> p (c t)"), ident[:])
    ptsb = work.tile([63, 128], F32, tag="tsb")
    nc.vector.tensor_copy(ptsb[:], ptp[:])
    nc.sync.dma_start(
        out=rTp[0:3, :].rearrange("c (t p) -> c t p", p=128), in_=ptsb[:]
    )
    sptp = mpsum.tile([21, 128], F32, tag="tp")
    nc.tensor.transpose(sptp[:], rsqp[:], ident[:])
    sptsb = work.tile([21, 128], F32, tag="tsb")
    nc.vector.tensor_copy(sptsb[:], sptp[:])
    nc.sync.dma_start(out=rTp[3:4, :], in_=sptsb[:])

    # ---------------- faces: corner indices, lo/hi decomposition ----------
    faces = sing.tile([128, FCH, 3], I32)
    nc.sync.dma_start(
        out=faces[:], in_=gf[:, :].rearrange("(p ch) w -> p ch w", p=128)
    )
    lo_i = sing.tile([128, FCH, 3], I32)
    hi_i = sing.tile([128, FCH, 3], I32)
    nc.vector.tensor_scalar(
        out=lo_i[:], in0=faces[:], scalar1=127, scalar2=None, op0=A.bitwise_and
    )
    nc.vector.tensor_scalar(
        out=hi_i[:], in0=faces[:], scalar1=7, scalar2=None, op0=A.logical_shift_right
    )
    lo_f = sing.tile([128, FCH, 3], F32)
    hi_f = sing.tile([128, FCH, 3], F32)
    nc.vector.tensor_copy(lo_f[:], lo_i[:])
    nc.vector.tensor_copy(hi_f[:], hi_i[:])

    # ---------------- iotas ----------------
    io128_i = sing.tile([128, 128], I32)
    nc.gpsimd.iota(io128_i[:], pattern=[[1, 128]], base=0, channel_multiplier=0)
    io128 = sing.tile([128, 128], F16)
    nc.vector.tensor_copy(io128[:], io128_i[:])
    io64_i = sing.tile([128, 64], I32)
    nc.gpsimd.iota(io64_i[:], pattern=[[1, 64]], base=0, channel_multiplier=0)
    io64 = sing.tile([128, 64], F16)
    nc.vector.tensor_copy(io64[:], io64_i[:])
    iogi = pkp.tile([128, NGT], I32, tag="pk")
    nc.gpsimd.iota(iogi[:], pattern=[[1, NGT]], base=0, channel_multiplier=0)
    iof = sing.tile([128, NGT], F32)
    nc.vector.tensor_scalar(
        out=iof[:], in0=iogi[:], scalar1=IOS, scalar2=None, op0=A.mult
    )

    # ---------------- cross products from host-gathered corners -----------
    eA = sing.tile([128, FCH, 3], F32)
    eB = sing.tile([128, FCH, 3], F32)
    nc.vector.tensor_tensor(
        out=eA[:], in0=crnT[:, :, 1, :], in1=crnT[:, :, 0, :], op=A.subtract
    )
    nc.vector.tensor_tensor(
        out=eB[:], in0=crnT[:, :, 2, :], in1=crnT[:, :, 0, :], op=A.subtract
    )
    fn = sing.tile([128, FCH, 3], F32)
    for d in range(3):
        u, v = (d + 1) % 3, (d + 2) % 3
        t1 = work.tile([128, FCH], F32, tag="cr1")
        t2 = work.tile([128, FCH], F32, tag="cr2")
        nc.vector.tensor_tensor(out=t1[:], in0=eA[:, :, u], in1=eB[:, :, v], op=A.mult)
        nc.vector.tensor_tensor(out=t2[:], in0=eA[:, :, v], in1=eB[:, :, u], op=A.mult)
        nc.vector.tensor_tensor(out=fn[:, :, d], in0=t1[:], in1=t2[:], op=A.subtract)

    # ---------------- one-hot scatter state: G[lo, c, hi] -----------------
    Gp = gpsum.tile([128, 3, 64], F32)
    _oh_state = {"k": 0}

    def emit_onehot(n):
        for _ in range(n):
            k = _oh_state["k"]
            if k >= 3 * FCH:
                return
            ch, corner = divmod(k, 3)
            ohlo = oh.tile([128, 128], F16, tag="ohlo")
            nc.vector.tensor_scalar(
                out=ohlo[:],
                in0=io128[:],
                scalar1=lo_f[:, ch : ch + 1, corner : corner + 1],
                scalar2=None,
                op0=A.is_equal,
            )
            R = oh.tile([128, 3, 64], F16, tag="R")
            for d in range(3):
                nc.gpsimd.tensor_scalar(
                    out=R[:, d, :],
                    in0=io64[:],
                    scalar1=hi_f[:, ch : ch + 1, corner : corner + 1],
                    scalar2=fn[:, ch : ch + 1, d : d + 1],
                    op0=A.is_equal,
                    op1=A.mult,
                )
            nc.tensor.matmul(
                Gp[:],
                ohlo[:],
                R[:],
                start=(k == 0),
                stop=(k == 3 * FCH - 1),
                skip_group_check=True,
            )
            _oh_state["k"] = k + 1

    # ---------------- search machinery ----------------
    pkmax_gt = sing.tile([128, PC], F32)
    pkmax_pr = sing.tile([128, PC], F32)

    def search_qchunk(i, rT_t, ncols, groups, evrow, pkmax_out):
        qTi = qT[:, i * 128 : (i + 1) * 128].bitcast(F32R)
        samp = work.tile([128, 1], F32, tag="samp")
        bias = work.tile([128, 1], F32, tag="bias")
        first = True
        for g0, g1 in groups:
            w = g1 - g0
            ps = spsum.tile([128, 1536], F32, tag="s")
            for c0 in range(0, w, 512):
                cw = min(512, w - c0)
                nc.tensor.matmul(
                    ps[:, c0 : c0 + cw],
                    qTi,
                    rT_t[:, g0 + c0 : g0 + c0 + cw].bitcast(F32R),
                    start=True,
                    stop=True,
                )
            if first:
                # row-max estimate from a strided sample of group 0
                sj = work.tile([128, w // 8], F32, tag="sj")
                nc.gpsimd.tensor_scalar(
                    out=sj[:], in0=ps[:, 0:w:8], scalar1=NEG, scalar2=None,
                    op0=A.max, op1=A.max, accum_out=samp[:],
                )
                # bias = 1024 + (1 - m^)*256
                nc.vector.tensor_scalar(
                    out=bias[:], in0=samp[:], scalar1=-WSC, scalar2=1024.0 + WSC,
                    op0=A.mult, op1=A.add,
                )
                first = False
            nc.scalar.activation(
                evrow[:, g0:g1], ps[:, 0:w], AF.Relu, bias=bias[:], scale=WSC
            )
        pk = pkp.tile([128, NGT], F32, tag="pk")
        nc.vector.tensor_tensor(
            out=pk[:, 0:ncols], in0=evrow[:], in1=iof[:, 0:ncols], op=A.add
        )
        nc.vector.tensor_scalar(
            out=pk[:, 0:ncols], in0=pk[:, 0:ncols], scalar1=NEG, scalar2=None,
            op0=A.max, op1=A.max, accum_out=pkmax_out[:, i : i + 1],
        )

    # gt-group chunk boundaries in columns
    gtg = [(a * 512, b * 512) for a, b in GT_GROUPS]

    # ---------------- main interleaved loop ----------------
    for i in range(PC):
        evp_p = evp.tile([128, VPAD], F16, tag="evp")
        search_qchunk(i, rTp, VPAD, PR_GROUPS, evp_p, pkmax_pr)
        emit_onehot(20)
    for i in range(PC):
        evg = evp.tile([128, NGT], F16, tag="evg")
        search_qchunk(i, rT, NGT, gtg, evg, pkmax_gt)
        emit_onehot(4)
        if dbg and i == 0:
            nc.sync.dma_start(out=dbg["dbg_evg"][:, :], in_=evg[:])
    emit_onehot(3 * FCH)  # leftovers

    # ---------------- decode pk -> column j -> vertex id ----------------
    def decode(pkmax_t, idx_t, mult):
        jf = work.tile([128, PC], F32, tag="jf")
        nc.vector.tensor_scalar(
            out=jf[:], in0=pkmax_t[:], scalar1=8192.0, scalar2=None, op0=A.mult
        )
        ji = work.tile([128, PC], I32, tag="ji")
        nc.vector.tensor_copy(ji[:], jf[:])
        nc.vector.tensor_scalar(
            out=ji[:], in0=ji[:], scalar1=8191, scalar2=None, op0=A.bitwise_and
        )
        # vertex = (j&127)*mult + (j>>7)
        a = work.tile([128, PC], I32, tag="ua")
        bcol = work.tile([128, PC], I32, tag="ub")
        nc.vector.tensor_scalar(
            out=a[:], in0=ji[:], scalar1=127, scalar2=mult, op0=A.bitwise_and,
            op1=A.mult,
        )
        nc.vector.tensor_scalar(
            out=bcol[:], in0=ji[:], scalar1=7, scalar2=None,
            op0=A.logical_shift_right,
        )
        nc.vector.tensor_tensor(out=idx_t[:], in0=a[:], in1=bcol[:], op=A.add)

    idx_gt = sing.tile([128, PC], I32)
    idx_pr = sing.tile([128, PC], I32)
    decode(pkmax_gt, idx_gt, 64)
    decode(pkmax_pr, idx_pr, 21)

    # ---------------- G -> DRAM (queue-split) ----------------
    Gs = sing.tile([128, 3, 64], F32)
    nc.scalar.copy(Gs[:], Gp[:])
    Gs2 = sing.tile([128, 64, 3], F32)
    nc.vector.tensor_copy(Gs2[:], Gs[:].rearrange("p c h -> p h c"))
    g_r = g_dram[:, :].rearrange(
        "(lo hi c) one -> lo (hi c one)", lo=128, hi=64
    )
    for part in range(8):
        nc.sync.dma_start(
            out=g_r[:, part * 24:(part + 1) * 24],
            in_=Gs2[:].rearrange("p h c -> p (h c)")[:, part * 24:(part + 1) * 24],
        )

    # gather offsets for normals: (v & 127)*192 + (v >> 7)*3
    o1 = work.tile([128, PC], I32, tag="o1")
    o2 = work.tile([128, PC], I32, tag="o2")
    nc.vector.tensor_scalar(
        out=o1[:], in0=idx_gt[:], scalar1=127, scalar2=192, op0=A.bitwise_and,
        op1=A.mult,
    )
    nc.vector.tensor_scalar(
        out=o2[:], in0=idx_gt[:], scalar1=7, scalar2=3,
        op0=A.logical_shift_right, op1=A.mult,
    )
    offs = sing.tile([128, PC], I32)
    nc.vector.tensor_tensor(out=offs[:], in0=o1[:], in1=o2[:], op=A.add)

    nGT = sing.tile([128, PC, 3], F32)
    for part in range(4):
        nc.gpsimd.indirect_dma_start(
            out=nGT[:, part * 4:(part + 1) * 4, :],
            out_offset=None,
            in_=g_dram[:, :],
            in_offset=bass.IndirectOffsetOnAxis(
                ap=offs[:, part * 4:(part + 1) * 4], axis=0
            ),
        )

    # nearest pred vertices
    idx_pr3 = sing.tile([128, PC], I32)
    nc.vector.tensor_scalar(
        out=idx_pr3[:], in0=idx_pr[:], scalar1=3, scalar2=None, op0=A.mult
    )
    pv_flat2 = pv[:, :].rearrange("v (c one) -> (v c) one", one=1)
    vNN = sing.tile([128, PC, 3], F32)
    for part in range(4):
        nc.gpsimd.indirect_dma_start(
            out=vNN[:, part * 4:(part + 1) * 4, :],
            out_offset=None,
            in_=pv_flat2,
            in_offset=bass.IndirectOffsetOnAxis(
                ap=idx_pr3[:, part * 4:(part + 1) * 4], axis=0
            ),
        )

    # ---------------- epilogue ----------------
    e = sing.tile([128, PC, 3], F32)
    nc.vector.tensor_tensor(out=e[:], in0=qRM[:], in1=vNN[:], op=A.subtract)
    tmp3 = work.tile([128, PC, 3], F32, tag="en")
    nc.vector.tensor_tensor(out=tmp3[:], in0=e[:], in1=nGT[:], op=A.mult)
    dot = sing.tile([128, PC], F32)
    nc.vector.tensor_reduce(out=dot[:], in_=tmp3[:], axis=AX.X, op=A.add)
    ee_t = work.tile([128, PC, 3], F32, tag="en")
    nc.vector.tensor_tensor(out=ee_t[:], in0=e[:], in1=e[:], op=A.mult)
    ee = sing.tile([128, PC], F32)
    nc.vector.tensor_reduce(out=ee[:], in_=ee_t[:], axis=AX.X, op=A.add)
    nn_t = work.tile([128, PC, 3], F32, tag="en")
    nc.vector.tensor_tensor(out=nn_t[:], in0=nGT[:], in1=nGT[:], op=A.mult)
    nn = sing.tile([128, PC], F32)
    nc.vector.tensor_reduce(out=nn[:], in_=nn_t[:], axis=AX.X, op=A.add)

    elen = sing.tile([128, PC], F32)
    nlen = sing.tile([128, PC], F32)
    nc.scalar.activation(elen[:], ee[:], AF.Sqrt)
    nc.scalar.activation(nlen[:], nn[:], AF.Sqrt)
    nc.vector.tensor_scalar(
        out=elen[:], in0=elen[:], scalar1=EPS, scalar2=None, op0=A.max
    )
    nc.vector.tensor_scalar(
        out=nlen[:], in0=nlen[:], scalar1=EPS, scalar2=None, op0=A.max
    )
    den = sing.tile([128, PC], F32)
    nc.vector.tensor_tensor(out=den[:], in0=elen[:], in1=nlen[:], op=A.mult)
    rden = sing.tile([128, PC], F32)
    nc.vector.reciprocal(rden[:], den[:])
    res = sing.tile([128, PC], F32)
    nc.vector.tensor_tensor(out=res[:], in0=dot[:], in1=rden[:], op=A.mult)
    partial = sing.tile([128, 1], F32)
    nc.vector.tensor_reduce(
        out=partial[:], in_=res[:], axis=AX.X, op=A.add, apply_absolute_value=True
    )
    ones = sing.tile([128, 1], F32)
    nc.vector.memset(ones[:], 1.0)
    fps = mpsum.tile([1, 1], F32, tag="tp")
    nc.tensor.matmul(fps[:], ones[:], partial[:], start=True, stop=True)
    osb = sing.tile([1, 1], F32)
    nc.scalar.copy(osb[:], fps[:])
    nc.sync.dma_start(out=out_dram[:], in_=osb[:])
    if dbg:
        nc.sync.dma_start(out=dbg["dbg_idx_gt"][:, :], in_=idx_gt[:])
        nc.sync.dma_start(out=dbg["dbg_idx_pr"][:, :], in_=idx_pr[:])
        nc.sync.dma_start(out=dbg["dbg_pkg"][:, :], in_=pkmax_gt[:])
        nc.sync.dma_start(out=dbg["dbg_pkp"][:, :], in_=pkmax_pr[:])
        nc.sync.dma_start(out=dbg["dbg_res"][:, :], in_=res[:])
        nc.sync.dma_start(out=dbg["dbg_g"][:, :], in_=Gs2[:].rearrange("p a b -> p (a b)"))
        nc.sync.dma_start(out=dbg["dbg_fn"][:, :], in_=fn[:].rearrange("p a b -> p (a b)"))


_NC_CACHE = None


def _get_nc():
    global _NC_CACHE
    if _NC_CACHE is None:
        _NC_CACHE = build_nc()
    return _NC_CACHE


def make_in_maps(pred_points, pred_vertices, gt_vertices, gt_faces):
    nb = pred_points.shape[0]
    faces32 = np.asarray(gt_faces).astype(np.int32, copy=False)
    out = []
    for b in range(nb):
        gvb = np.ascontiguousarray(gt_vertices[b], dtype=np.float32)
        fb = np.ascontiguousarray(faces32[b])
        out.append(
            dict(
                pred_points=np.ascontiguousarray(pred_points[b], dtype=np.float32),
                pred_vertices=np.ascontiguousarray(pred_vertices[b], dtype=np.float32),
                gt_vertices=gvb,
                gt_faces32=fb,
                corners=np.ascontiguousarray(gvb[fb]),  # [NF, 3, 3]
            )
        )
    return out


def kernel(pred_points, pred_vertices, gt_vertices, gt_faces):
    from concourse.bass_utils import run_bass_kernel_spmd

    nb = pred_points.shape[0]
    nc = _get_nc()
    in_maps = make_in_maps(pred_points, pred_vertices, gt_vertices, gt_faces)
    res = run_bass_kernel_spmd(nc, in_maps, list(range(nb)))
    total = sum(float(res.results[i]["loss_sum"][0]) for i in range(nb))
    return np.array(total / (nb * P), dtype=np.float32)


if __name__ == "__main__":
    nc = build_nc()
    print("built ok")


# revision 6
# speedup vs baseline: 1.1005x; 1.0441x over previous
"""Trainium2 Bass kernel for nn_ChamferNormalLoss (8-core data parallel).

Sharding: pure data parallel - one batch sample per NeuronCore; the host
averages the 8 per-core |dot| sums (the only cross-core reduction).

Per-sample pipeline on each core (v2):
  1. NN searches as TensorE float32r matmuls (1 cyc/row vs fp32's 4) with a
     K=4 contraction fusing the bias: s = 2*q.r - |r|^2.  Reduction via a
     windowed fp16 pack: a strided GPSIMD sample pass over PSUM gives a
     row-max estimate m^; ScalarE evacuates PSUM with
     relu((s - m^ + 1)*2^8 + 1024) to fp16, which lands winners in
     [1024,2048) where fp16 ulp=1 rounds scores to integers for free; one
     VectorE tensor_tensor adds iota*2^-13 (span < 1 = the fp16 ulp, so
     score order can never be violated) into an fp32 pk and a 2x max-accum
     pass yields pk_max = q* + j*2^-13; j* decodes with integer ops.
     Quantization (2^-8 in score units) flips only near-tie neighbours,
     which leaves the mean |dot| loss unchanged to ~1e-3 (measured).
  2. Face corner coordinates are host-pregathered (gv[faces] is pure input
     prep); cross products on VectorE; the scatter-add n[v] += fn is
     factorized via v = hi*128 + lo: per (face-chunk, corner) a one-hot(lo)
     [128f,128lo] fp16 (VectorE) is the matmul weight and one-hot(hi)*fn
     [128f,3*64] fp16 (GPSIMD) the moving operand; one PSUM tile
     accumulates G[lo,c,hi] over all 384 chunk-corner pairs, interleaved
     with the search chunks to keep all engines busy.
  3. Epilogue: batched multi-column indirect row-gathers of nearest gt
     normals (from G in DRAM) and nearest pred vertices; |dot| via
     dot/(max(|e|,eps)*max(|n|,eps)); abs-sum reduce; partition sum via a
     ones-matmul.
"""

import os, sys

for _p in (
    "/opt/trn_rl_repo",
    "/opt/pypackages",
    "/root/.axon_site/_ro/trn_rl_repo",
    "/root/.axon_site/_ro/pypackages",
):
    if os.path.isdir(_p) and _p not in sys.path:
        sys.path.insert(0, _p)

import numpy as np

import concourse.bass as bass
import concourse.bacc as bacc
import concourse.tile as tile
from concourse import masks, mybir

F32 = mybir.dt.float32
F16 = mybir.dt.float16
F32R = mybir.dt.float32r
I32 = mybir.dt.int32
A = mybir.AluOpType
AF = mybir.ActivationFunctionType
AX = mybir.AxisListType

B = 8
P, PC = 2048, 16          # queries, chunks of 128
NGT = 8192                # gt vertices; searched as 16 chunks of 512
VPR, VPAD = 2562, 2688    # pred vertices, padded to 21*128
NF, FCH = 16384, 128      # faces, chunks of 128
BIGC = 1.0e6              # pad coordinate; score approx -3e12

EPS = 1e-12
NEG = -3.0e38
WSC = 256.0               # 2^8 window scale
IOS = 2.0 ** -13          # iota step

# gt groups: 16 chunks of 512 -> PSUM groups of (3,3,3,3,3,1) chunks
GT_GROUPS = [(0, 3), (3, 6), (6, 9), (9, 12), (12, 15), (15, 16)]
# pred: 2688 cols = chunks (512*5 + 128) -> groups of (3 chunks, 2+tail)
PR_GROUPS = [(0, 1536), (1536, 2688)]


def build_nc(debug_outs=False):
    nc = bacc.Bacc(None, target_bir_lowering=False)
    pp = nc.dram_tensor("pred_points", [P, 3], F32, kind="ExternalInput")
    pv = nc.dram_tensor("pred_vertices", [VPR, 3], F32, kind="ExternalInput")
    gv = nc.dram_tensor("gt_vertices", [NGT, 3], F32, kind="ExternalInput")
    gf = nc.dram_tensor("gt_faces32", [NF, 3], I32, kind="ExternalInput")
    crn = nc.dram_tensor("corners", [NF, 3, 3], F32, kind="ExternalInput")
    g_dram = nc.dram_tensor("g_norm", [NGT * 3, 1], F32)
    out = nc.dram_tensor("loss_sum", [1], F32, kind="ExternalOutput")

    from contextlib import ExitStack

    dbg = {}
    if debug_outs:
        for nm, shape, dt in [
            ("dbg_idx_gt", [128, PC], I32), ("dbg_idx_pr", [128, PC], I32),
            ("dbg_pkg", [128, PC], F32), ("dbg_pkp", [128, PC], F32),
            ("dbg_res", [128, PC], F32), ("dbg_g", [128, 192], F32),
            ("dbg_evg", [128, NGT], F16), ("dbg_fn", [128, FCH * 3], F32),
        ]:
            dbg[nm] = nc.dram_tensor(nm, shape, dt, kind="ExternalOutput")
    with tile.TileContext(nc) as tc, ExitStack() as ctx:
        _body(nc, tc, ctx, pp, pv, gv, gf, crn, g_dram, out, dbg)
    nc.compile()
    return nc


def _body(nc, tc, ctx, pp, pv, gv, gf, crn, g_dram, out_dram, dbg=None):
    sing = ctx.enter_context(tc.tile_pool(name="sing", bufs=1))
    work = ctx.enter_context(tc.tile_pool(name="work", bufs=2))
    oh = ctx.enter_context(tc.tile_pool(name="oh", bufs=3))
    evp = ctx.enter_context(tc.tile_pool(name="evp", bufs=2))
    pkp = ctx.enter_context(tc.tile_pool(name="pkp", bufs=1))
    spsum = ctx.enter_context(
        tc.tile_pool(name="spsum", bufs=2, space=bass.MemorySpace.PSUM)
    )
    mpsum = ctx.enter_context(
        tc.tile_pool(name="mpsum", bufs=1, space=bass.MemorySpace.PSUM)
    )
    gpsum = ctx.enter_context(
        tc.tile_pool(name="gpsum", bufs=1, space=bass.MemorySpace.PSUM)
    )

    ident0 = sing.tile([128, 128], F32)
    masks.make_identity(nc, ident0[:])
    ident = sing.tile([128, 128], F32)
    nc.vector.tensor_copy(ident[:], ident0[:])

    # ---- corners: [16384,3,3] -> [128, FCH, 3, 3], face f = p*FCH + ch
    crnT = sing.tile([128, FCH, 3, 3], F32)
    crn_r = crn[:, :, :].rearrange("(p ch) c d -> p ch c d", p=128)
    for part in range(8):
        nc.sync.dma_start(
            out=crnT[:, part * 16:(part + 1) * 16, :, :],
            in_=crn_r[:, part * 16:(part + 1) * 16, :, :],
        )

    # ---- query side: qT[:, n] = [2qx,2qy,2qz,-1], query (n&127)*16 + (n>>7)
    qRM = sing.tile([128, PC, 3], F32)
    nc.sync.dma_start(out=qRM[:], in_=pp[:, :].rearrange("(p i) c -> p i c", p=128))
    qCM = work.tile([128, 3, PC], F32, tag="qcm")
    nc.vector.tensor_copy(qCM[:], qRM[:].rearrange("p i c -> p c i"))
    qT = sing.tile([4, P], F32)
    nc.vector.memset(qT[:, :], -1.0)
    qtp = mpsum.tile([48, 128], F32, tag="tp")
    nc.tensor.transpose(qtp[:], qCM[:].rearrange("p c i -> p (c i)"), ident[:])
    qtsb = work.tile([48, 128], F32, tag="tsb")
    nc.vector.tensor_scalar(
        out=qtsb[:], in0=qtp[:], scalar1=2.0, scalar2=None, op0=A.mult
    )
    nc.sync.dma_start(
        out=qT[0:3, :].rearrange("c (i p) -> c i p", p=128), in_=qtsb[:]
    )

    # ---- gt side: rT[:, n] = [x,y,z,|r|^2], vertex (n&127)*64 + (n>>7)
    rRM = work.tile([128, 64, 3], F32, tag="rrm")
    nc.sync.dma_start(out=rRM[:], in_=gv[:, :].rearrange("(p t) c -> p t c", p=128))
    rCM = work.tile([128, 2, 3, 32], F32, tag="rcm")
    nc.vector.tensor_copy(rCM[:], rRM[:].rearrange("p (h t) c -> p h c t", h=2))
    sq = work.tile([128, 64, 3], F32, tag="sq")
    nc.vector.tensor_tensor(out=sq[:], in0=rRM[:], in1=rRM[:], op=A.mult)
    rsq = work.tile([128, 64], F32, tag="rsq")
    nc.vector.tensor_reduce(out=rsq[:], in_=sq[:], axis=AX.X, op=A.add)
    rT = sing.tile([4, NGT], F32)
    for h in range(2):
        ctp = mpsum.tile([96, 128], F32, tag="tp")
        nc.tensor.transpose(
            ctp[:], rCM[:, h, :, :].rearrange("p c t -> p (c t)"), ident[:]
        )
        ctsb = work.tile([96, 128], F32, tag="tsb")
        nc.vector.tensor_copy(ctsb[:], ctp[:])
        nc.sync.dma_start(
            out=rT[0:3, h * 32 * 128 : (h + 1) * 32 * 128].rearrange(
                "c (t p) -> c t p", p=128
            ),
            in_=ctsb[:],
        )
    stp = mpsum.tile([64, 128], F32, tag="tp")
    nc.tensor.transpose(stp[:], rsq[:], ident[:])
    stsb = work.tile([64, 128], F32, tag="tsb")
    nc.vector.tensor_copy(stsb[:], stp[:])
    nc.sync.dma_start(out=rT[3:4, :], in_=stsb[:])

    # ---- pred side (padded to 2688): vertex (n&127)*21 + (n>>7)
    rRMp = work.tile([128, 21, 3], F32, tag="rrmp")
    nc.vector.memset(rRMp[:], BIGC)
    rRMp_f = rRMp[:].rearrange("p t c -> p (t c)")
    pv_f = pv[:, :].rearrange("v c -> (v c)")
    nc.sync.dma_start(
        out=rRMp_f[0:122, :],
        in_=pv_f[0 : 122 * 63].rearrange("(p a) -> p a", a=63),
    )
    rCMp = work.tile([128, 3, 21], F32, tag="rcmp")
    nc.vector.tensor_copy(rCMp[:], rRMp[:].rearrange("p t c -> p c t"))
    sqp = work.tile([128, 21, 3], F32, tag="sqp")
    nc.vector.tensor_tensor(out=sqp[:], in0=rRMp[:], in1=rRMp[:], op=A.mult)
    rsqp = work.tile([128, 21], F32, tag="rsqp")
    nc.vector.tensor_reduce(out=rsqp[:], in_=sqp[:], axis=AX.X, op=A.add)
    rTp = sing.tile([4, VPAD], F32)
    ptp = mpsum.tile([63, 128], F32, tag="tp")
    nc.tensor.transpose(ptp[:], rCMp[:].rearrange("p c t -> p (c t)"), ident[:])
    ptsb = work.tile([63, 128], F32, tag="tsb")
    nc.vector.tensor_copy(ptsb[:], ptp[:])
    nc.sync.dma_start(
        out=rTp[0:3, :].rearrange("c (t p) -> c t p", p=128), in_=ptsb[:]
    )
    sptp = mpsum.tile([21, 128], F32, tag="tp")
    nc.tensor.transpose(sptp[:], rsqp[:], ident[:])
    sptsb = work.tile([21, 128], F32, tag="tsb")
    nc.vector.tensor_copy(sptsb[:], sptp[:])
    nc.sync.dma_start(out=rTp[3:4, :], in_=sptsb[:])

    # ---------------- faces: corner indices, lo/hi decomposition ----------
    faces = sing.tile([128, FCH, 3], I32)
    nc.sync.dma_start(
        out=faces[:], in_=gf[:, :].rearrange("(p ch) w -> p ch w", p=128)
    )
    lo_i = sing.tile([128, FCH, 3], I32)
    hi_i = sing.tile([128, FCH, 3], I32)
    nc.vector.tensor_scalar(
        out=lo_i[:], in0=faces[:], scalar1=127, scalar2=None, op0=A.bitwise_and
    )
    nc.vector.tensor_scalar(
        out=hi_i[:], in0=faces[:], scalar1=7, scalar2=None, op0=A.logical_shift_right
    )
    lo_f = sing.tile([128, FCH, 3], F32)
    hi_f = sing.tile([128, FCH, 3], F32)
    nc.vector.tensor_copy(lo_f[:], lo_i[:])
    nc.vector.tensor_copy(hi_f[:], hi_i[:])

    # ---------------- iotas ----------------
    io128_i = sing.tile([128, 128], I32)
    nc.gpsimd.iota(io128_i[:], pattern=[[1, 128]], base=0, channel_multiplier=0)
    io128 = sing.tile([128, 128], F16)
    nc.vector.tensor_copy(io128[:], io128_i[:])
    io64_i = sing.tile([128, 64], I32)
    nc.gpsimd.iota(io64_i[:], pattern=[[1, 64]], base=0, channel_multiplier=0)
    io64 = sing.tile([128, 64], F16)
    nc.vector.tensor_copy(io64[:], io64_i[:])
    iogi = pkp.tile([128, NGT], I32, tag="pk")
    nc.gpsimd.iota(iogi[:], pattern=[[1, NGT]], base=0, channel_multiplier=0)
    iof = sing.tile([128, NGT], F32)
    nc.vector.tensor_scalar(
        out=iof[:], in0=iogi[:], scalar1=IOS, scalar2=None, op0=A.mult
    )

    # ---------------- cross products from host-gathered corners -----------
    eA = sing.tile([128, FCH, 3], F32)
    eB = sing.tile([128, FCH, 3], F32)
    nc.vector.tensor_tensor(
        out=eA[:], in0=crnT[:, :, 1, :], in1=crnT[:, :, 0, :], op=A.subtract
    )
    nc.vector.tensor_tensor(
        out=eB[:], in0=crnT[:, :, 2, :], in1=crnT[:, :, 0, :], op=A.subtract
    )
    fn = sing.tile([128, FCH, 3], F32)
    for d in range(3):
        u, v = (d + 1) % 3, (d + 2) % 3
        t1 = work.tile([128, FCH], F32, tag="cr1")
        t2 = work.tile([128, FCH], F32, tag="cr2")
        nc.vector.tensor_tensor(out=t1[:], in0=eA[:, :, u], in1=eB[:, :, v], op=A.mult)
        nc.vector.tensor_tensor(out=t2[:], in0=eA[:, :, v], in1=eB[:, :, u], op=A.mult)
        nc.vector.tensor_tensor(out=fn[:, :, d], in0=t1[:], in1=t2[:], op=A.subtract)

    # ---------------- one-hot scatter state: G[lo, c, hi] -----------------
    Gp = gpsum.tile([128, 3, 64], F32)
    _oh_state = {"k": 0}

    def emit_onehot(n):
        for _ in range(n):
            k = _oh_state["k"]
            if k >= 3 * FCH:
                return
            ch, corner = divmod(k, 3)
            ohlo = oh.tile([128, 128], F16, tag="ohlo")
            nc.vector.tensor_scalar(
                out=ohlo[:],
                in0=io128[:],
                scalar1=lo_f[:, ch : ch + 1, corner : corner + 1],
                scalar2=None,
                op0=A.is_equal,
            )
            R = oh.tile([128, 3, 64], F16, tag="R")
            for d in range(3):
                nc.gpsimd.tensor_scalar(
                    out=R[:, d, :],
                    in0=io64[:],
                    scalar1=hi_f[:, ch : ch + 1, corner : corner + 1],
                    scalar2=fn[:, ch : ch + 1, d : d + 1],
                    op0=A.is_equal,
                    op1=A.mult,
                )
            nc.tensor.matmul(
                Gp[:],
                ohlo[:],
                R[:],
                start=(k == 0),
                stop=(k == 3 * FCH - 1),
                skip_group_check=True,
            )
            _oh_state["k"] = k + 1

    # ---------------- search machinery ----------------
    pkmax_gt = sing.tile([128, PC], F32)
    pkmax_pr = sing.tile([128, PC], F32)

    def search_qchunk(i, rT_t, ncols, groups, evrow, pkmax_out):
        qTi = qT[:, i * 128 : (i + 1) * 128].bitcast(F32R)
        samp = work.tile([128, 1], F32, tag="samp")
        bias = work.tile([128, 1], F32, tag="bias")
        first = True
        for g0, g1 in groups:
            w = g1 - g0
            ps = spsum.tile([128, 1536], F32, tag="s")
            for c0 in range(0, w, 512):
                cw = min(512, w - c0)
                nc.tensor.matmul(
                    ps[:, c0 : c0 + cw],
                    qTi,
                    rT_t[:, g0 + c0 : g0 + c0 + cw].bitcast(F32R),
                    start=True,
                    stop=True,
                )
            if first:
                # row-max estimate from a strided sample of group 0
                sj = work.tile([128, w // 8], F32, tag="sj")
                nc.gpsimd.tensor_scalar(
                    out=sj[:], in0=ps[:, 0:w:8], scalar1=NEG, scalar2=None,
                    op0=A.max, op1=A.max, accum_out=samp[:],
                )
                # bias = 1024 + (1 - m^)*256
                nc.vector.tensor_scalar(
                    out=bias[:], in0=samp[:], scalar1=-WSC, scalar2=1024.0 + WSC,
                    op0=A.mult, op1=A.add,
                )
                first = False
            nc.scalar.activation(
                evrow[:, g0:g1], ps[:, 0:w], AF.Relu, bias=bias[:], scale=WSC
            )
        # pk pass split across DVE / GPSIMD (tensor_tensor is 1x on both;
        # Pool is 1.33x slower so it gets the smaller share)
        wA = 3584 if ncols == NGT else 1024
        pk = pkp.tile([128, NGT], F32, tag="pk")
        nc.vector.tensor_tensor(
            out=pk[:, 0:wA], in0=evrow[:, 0:wA], in1=iof[:, 0:wA], op=A.add
        )
        nc.gpsimd.tensor_tensor(
            out=pk[:, wA:ncols], in0=evrow[:, wA:ncols], in1=iof[:, wA:ncols],
            op=A.add,
        )
        nc.vector.tensor_scalar(
            out=pk[:, 0:ncols], in0=pk[:, 0:ncols], scalar1=NEG, scalar2=None,
            op0=A.max, op1=A.max, accum_out=pkmax_out[:, i : i + 1],
        )

    # gt-group chunk boundaries in columns
    gtg = [(a * 512, b * 512) for a, b in GT_GROUPS]

    # ---------------- main interleaved loop ----------------
    for i in range(PC):
        evg = evp.tile([128, NGT], F16, tag="evg")
        search_qchunk(i, rT, NGT, gtg, evg, pkmax_gt)
        emit_onehot(12)
        evp_p = evp.tile([128, VPAD], F16, tag="evp")
        search_qchunk(i, rTp, VPAD, PR_GROUPS, evp_p, pkmax_pr)
        emit_onehot(12)
        if dbg and i == 0:
            nc.sync.dma_start(out=dbg["dbg_evg"][:, :], in_=evg[:])
    emit_onehot(3 * FCH)  # leftovers

    # ---------------- decode pk -> column j -> vertex id ----------------
    def decode(pkmax_t, idx_t, mult):
        jf = work.tile([128, PC], F32, tag="jf")
        nc.vector.tensor_scalar(
            out=jf[:], in0=pkmax_t[:], scalar1=8192.0, scalar2=None, op0=A.mult
        )
        ji = work.tile([128, PC], I32, tag="ji")
        nc.vector.tensor_copy(ji[:], jf[:])
        nc.vector.tensor_scalar(
            out=ji[:], in0=ji[:], scalar1=8191, scalar2=None, op0=A.bitwise_and
        )
        # vertex = (j&127)*mult + (j>>7)
        a = work.tile([128, PC], I32, tag="ua")
        bcol = work.tile([128, PC], I32, tag="ub")
        nc.vector.tensor_scalar(
            out=a[:], in0=ji[:], scalar1=127, scalar2=mult, op0=A.bitwise_and,
            op1=A.mult,
        )
        nc.vector.tensor_scalar(
            out=bcol[:], in0=ji[:], scalar1=7, scalar2=None,
            op0=A.logical_shift_right,
        )
        nc.vector.tensor_tensor(out=idx_t[:], in0=a[:], in1=bcol[:], op=A.add)

    idx_gt = sing.tile([128, PC], I32)
    idx_pr = sing.tile([128, PC], I32)
    decode(pkmax_gt, idx_gt, 64)
    decode(pkmax_pr, idx_pr, 21)

    # ---------------- G -> DRAM (queue-split) ----------------
    Gs = sing.tile([128, 3, 64], F32)
    nc.scalar.copy(Gs[:], Gp[:])
    Gs2 = sing.tile([128, 64, 3], F32)
    nc.vector.tensor_copy(Gs2[:], Gs[:].rearrange("p c h -> p h c"))
    g_r = g_dram[:, :].rearrange(
        "(lo hi c) one -> lo (hi c one)", lo=128, hi=64
    )
    for part in range(8):
        nc.sync.dma_start(
            out=g_r[:, part * 24:(part + 1) * 24],
            in_=Gs2[:].rearrange("p h c -> p (h c)")[:, part * 24:(part + 1) * 24],
        )

    # gather offsets for normals: (v & 127)*192 + (v >> 7)*3
    o1 = work.tile([128, PC], I32, tag="o1")
    o2 = work.tile([128, PC], I32, tag="o2")
    nc.vector.tensor_scalar(
        out=o1[:], in0=idx_gt[:], scalar1=127, scalar2=192, op0=A.bitwise_and,
        op1=A.mult,
    )
    nc.vector.tensor_scalar(
        out=o2[:], in0=idx_gt[:], scalar1=7, scalar2=3,
        op0=A.logical_shift_right, op1=A.mult,
    )
    offs = sing.tile([128, PC], I32)
    nc.vector.tensor_tensor(out=offs[:], in0=o1[:], in1=o2[:], op=A.add)

    nGT = sing.tile([128, PC, 3], F32)
    for part in range(4):
        nc.gpsimd.indirect_dma_start(
            out=nGT[:, part * 4:(part + 1) * 4, :],
            out_offset=None,
            in_=g_dram[:, :],
            in_offset=bass.IndirectOffsetOnAxis(
                ap=offs[:, part * 4:(part + 1) * 4], axis=0
            ),
        )

    # nearest pred vertices
    idx_pr3 = sing.tile([128, PC], I32)
    nc.vector.tensor_scalar(
        out=idx_pr3[:], in0=idx_pr[:], scalar1=3, scalar2=None, op0=A.mult
    )
    pv_flat2 = pv[:, :].rearrange("v (c one) -> (v c) one", one=1)
    vNN = sing.tile([128, PC, 3], F32)
    for part in range(4):
        nc.gpsimd.indirect_dma_start(
            out=vNN[:, part * 4:(part + 1) * 4, :],
            out_offset=None,
            in_=pv_flat2,
            in_offset=bass.IndirectOffsetOnAxis(
                ap=idx_pr3[:, part * 4:(part + 1) * 4], axis=0
            ),
        )

    # ---------------- epilogue ----------------
    e = sing.tile([128, PC, 3], F32)
    nc.vector.tensor_tensor(out=e[:], in0=qRM[:], in1=vNN[:], op=A.subtract)
    tmp3 = work.tile([128, PC, 3], F32, tag="en")
    nc.vector.tensor_tensor(out=tmp3[:], in0=e[:], in1=nGT[:], op=A.mult)
    dot = sing.tile([128, PC], F32)
    nc.vector.tensor_reduce(out=dot[:], in_=tmp3[:], axis=AX.X, op=A.add)
    ee_t = work.tile([128, PC, 3], F32, tag="en")
    nc.vector.tensor_tensor(out=ee_t[:], in0=e[:], in1=e[:], op=A.mult)
    ee = sing.tile([128, PC], F32)
    nc.vector.tensor_reduce(out=ee[:], in_=ee_t[:], axis=AX.X, op=A.add)
    nn_t = work.tile([128, PC, 3], F32, tag="en")
    nc.vector.tensor_tensor(out=nn_t[:], in0=nGT[:], in1=nGT[:], op=A.mult)
    nn = sing.tile([128, PC], F32)
    nc.vector.tensor_reduce(out=nn[:], in_=nn_t[:], axis=AX.X, op=A.add)

    elen = sing.tile([128, PC], F32)
    nlen = sing.tile([128, PC], F32)
    nc.scalar.activation(elen[:], ee[:], AF.Sqrt)
    nc.scalar.activation(nlen[:], nn[:], AF.Sqrt)
    nc.vector.tensor_scalar(
        out=elen[:], in0=elen[:], scalar1=EPS, scalar2=None, op0=A.max
    )
    nc.vector.tensor_scalar(
        out=nlen[:], in0=nlen[:], scalar1=EPS, scalar2=None, op0=A.max
    )
    den = sing.tile([128, PC], F32)
    nc.vector.tensor_tensor(out=den[:], in0=elen[:], in1=nlen[:], op=A.mult)
    rden = sing.tile([128, PC], F32)
    nc.vector.reciprocal(rden[:], den[:])
    res = sing.tile([128, PC], F32)
    nc.vector.tensor_tensor(out=res[:], in0=dot[:], in1=rden[:], op=A.mult)
    partial = sing.tile([128, 1], F32)
    nc.vector.tensor_reduce(
        out=partial[:], in_=res[:], axis=AX.X, op=A.add, apply_absolute_value=True
    )
    ones = sing.tile([128, 1], F32)
    nc.vector.memset(ones[:], 1.0)
    fps = mpsum.tile([1, 1], F32, tag="tp")
    nc.tensor.matmul(fps[:], ones[:], partial[:], start=True, stop=True)
    osb = sing.tile([1, 1], F32)
    nc.scalar.copy(osb[:], fps[:])
    nc.sync.dma_start(out=out_dram[:], in_=osb[:])
    if dbg:
        nc.sync.dma_start(out=dbg["dbg_idx_gt"][:, :], in_=idx_gt[:])
        nc.sync.dma_start(out=dbg["dbg_idx_pr"][:, :], in_=idx_pr[:])
        nc.sync.dma_start(out=dbg["dbg_pkg"][:, :], in_=pkmax_gt[:])
        nc.sync.dma_start(out=dbg["dbg_pkp"][:, :], in_=pkmax_pr[:])
        nc.sync.dma_start(out=dbg["dbg_res"][:, :], in_=res[:])
        nc.sync.dma_start(out=dbg["dbg_g"][:, :], in_=Gs2[:].rearrange("p a b -> p (a b)"))
        nc.sync.dma_start(out=dbg["dbg_fn"][:, :], in_=fn[:].rearrange("p a b -> p (a b)"))


_NC_CACHE = None


def _get_nc():
    global _NC_CACHE
    if _NC_CACHE is None:
        _NC_CACHE = build_nc()
    return _NC_CACHE


def make_in_maps(pred_points, pred_vertices, gt_vertices, gt_faces):
    nb = pred_points.shape[0]
    faces32 = np.asarray(gt_faces).astype(np.int32, copy=False)
    out = []
    for b in range(nb):
        gvb = np.ascontiguousarray(gt_vertices[b], dtype=np.float32)
        fb = np.ascontiguousarray(faces32[b])
        out.append(
            dict(
                pred_points=np.ascontiguousarray(pred_points[b], dtype=np.float32),
                pred_vertices=np.ascontiguousarray(pred_vertices[b], dtype=np.float32),
                gt_vertices=gvb,
                gt_faces32=fb,
                corners=np.ascontiguousarray(gvb[fb]),  # [NF, 3, 3]
            )
        )
    return out


def kernel(pred_points, pred_vertices, gt_vertices, gt_faces):
    from concourse.bass_utils import run_bass_kernel_spmd

    nb = pred_points.shape[0]
    nc = _get_nc()
    in_maps = make_in_maps(pred_points, pred_vertices, gt_vertices, gt_faces)
    res = run_bass_kernel_spmd(nc, in_maps, list(range(nb)))
    total = sum(float(res.results[i]["loss_sum"][0]) for i in range(nb))
    return np.array(total / (nb * P), dtype=np.float32)


if __name__ == "__main__":
    nc = build_nc()
    print("built ok")


# revision 7
# speedup vs baseline: 1.1515x; 1.0463x over previous
"""Trainium2 Bass kernel for nn_ChamferNormalLoss (8-core data parallel).

Sharding: pure data parallel - one batch sample per NeuronCore; the host
averages the 8 per-core |dot| sums (the only cross-core reduction).

Per-sample pipeline on each core (v2):
  1. NN searches as TensorE float32r matmuls (1 cyc/row vs fp32's 4) with a
     K=4 contraction fusing the bias: s = 2*q.r - |r|^2.  Reduction via a
     windowed fp16 pack: a strided GPSIMD sample pass over PSUM gives a
     row-max estimate m^; ScalarE evacuates PSUM with
     relu((s - m^ + 1)*2^8 + 1024) to fp16, which lands winners in
     [1024,2048) where fp16 ulp=1 rounds scores to integers for free; one
     VectorE tensor_tensor adds iota*2^-13 (span < 1 = the fp16 ulp, so
     score order can never be violated) into an fp32 pk and a 2x max-accum
     pass yields pk_max = q* + j*2^-13; j* decodes with integer ops.
     Quantization (2^-8 in score units) flips only near-tie neighbours,
     which leaves the mean |dot| loss unchanged to ~1e-3 (measured).
  2. Face corner coordinates are host-pregathered (gv[faces] is pure input
     prep); cross products on VectorE; the scatter-add n[v] += fn is
     factorized via v = hi*128 + lo: per (face-chunk, corner) a one-hot(lo)
     [128f,128lo] fp16 (VectorE) is the matmul weight and one-hot(hi)*fn
     [128f,3*64] fp16 (GPSIMD) the moving operand; one PSUM tile
     accumulates G[lo,c,hi] over all 384 chunk-corner pairs, interleaved
     with the search chunks to keep all engines busy.
  3. Epilogue: batched multi-column indirect row-gathers of nearest gt
     normals (from G in DRAM) and nearest pred vertices; |dot| via
     dot/(max(|e|,eps)*max(|n|,eps)); abs-sum reduce; partition sum via a
     ones-matmul.
"""

import os, sys

for _p in (
    "/opt/trn_rl_repo",
    "/opt/pypackages",
    "/root/.axon_site/_ro/trn_rl_repo",
    "/root/.axon_site/_ro/pypackages",
):
    if os.path.isdir(_p) and _p not in sys.path:
        sys.path.insert(0, _p)

import numpy as np

import concourse.bass as bass
import concourse.bacc as bacc
import concourse.tile as tile
from concourse import masks, mybir

F32 = mybir.dt.float32
F16 = mybir.dt.float16
F32R = mybir.dt.float32r
I32 = mybir.dt.int32
A = mybir.AluOpType
AF = mybir.ActivationFunctionType
AX = mybir.AxisListType

B = 8
P, PC = 2048, 16          # queries, chunks of 128
NGT = 8192                # gt vertices; searched as 16 chunks of 512
VPR, VPAD = 2562, 2688    # pred vertices, padded to 21*128
NF, FCH = 16384, 128      # faces, chunks of 128
BIGC = 1.0e6              # pad coordinate; score approx -3e12

EPS = 1e-12
NEG = -3.0e38
WSC = 256.0               # 2^8 window scale
IOS = 2.0 ** -13          # iota step

# gt groups: 16 chunks of 512 -> PSUM groups of (3,3,3,3,3,1) chunks
GT_GROUPS = [(0, 3), (3, 6), (6, 9), (9, 12), (12, 15), (15, 16)]
# pred: 2688 cols = chunks (512*5 + 128) -> groups of (3 chunks, 2+tail)
PR_GROUPS = [(0, 1536), (1536, 2688)]


def build_nc(debug_outs=False):
    nc = bacc.Bacc(None, target_bir_lowering=False)
    pp = nc.dram_tensor("pred_points", [P, 3], F32, kind="ExternalInput")
    pv = nc.dram_tensor("pred_vertices", [VPR, 3], F32, kind="ExternalInput")
    gv = nc.dram_tensor("gt_vertices", [NGT, 3], F32, kind="ExternalInput")
    gf = nc.dram_tensor("gt_faces32", [NF, 3], I32, kind="ExternalInput")
    crn = nc.dram_tensor("corners", [NF, 3, 3], F32, kind="ExternalInput")
    g_dram = nc.dram_tensor("g_norm", [NGT * 3, 1], F32)
    out = nc.dram_tensor("loss_sum", [1], F32, kind="ExternalOutput")

    from contextlib import ExitStack

    dbg = {}
    if debug_outs:
        for nm, shape, dt in [
            ("dbg_idx_gt", [128, PC], I32), ("dbg_idx_pr", [128, PC], I32),
            ("dbg_pkg", [128, PC], F32), ("dbg_pkp", [128, PC], F32),
            ("dbg_res", [128, PC], F32), ("dbg_g", [128, 192], F32),
            ("dbg_evg", [128, NGT], F16), ("dbg_fn", [128, FCH * 3], F32),
        ]:
            dbg[nm] = nc.dram_tensor(nm, shape, dt, kind="ExternalOutput")
    with tile.TileContext(nc) as tc, ExitStack() as ctx:
        _body(nc, tc, ctx, pp, pv, gv, gf, crn, g_dram, out, dbg)
    nc.compile()
    return nc


def _body(nc, tc, ctx, pp, pv, gv, gf, crn, g_dram, out_dram, dbg=None):
    sing = ctx.enter_context(tc.tile_pool(name="sing", bufs=1))
    work = ctx.enter_context(tc.tile_pool(name="work", bufs=2))
    oh = ctx.enter_context(tc.tile_pool(name="oh", bufs=9))
    evp = ctx.enter_context(tc.tile_pool(name="evp", bufs=2))
    pkp = ctx.enter_context(tc.tile_pool(name="pkp", bufs=1))
    spsum = ctx.enter_context(
        tc.tile_pool(name="spsum", bufs=2, space=bass.MemorySpace.PSUM)
    )
    mpsum = ctx.enter_context(
        tc.tile_pool(name="mpsum", bufs=1, space=bass.MemorySpace.PSUM)
    )
    gpsum = ctx.enter_context(
        tc.tile_pool(name="gpsum", bufs=1, space=bass.MemorySpace.PSUM)
    )

    ident0 = sing.tile([128, 128], F32)
    masks.make_identity(nc, ident0[:])
    ident = sing.tile([128, 128], F32)
    nc.vector.tensor_copy(ident[:], ident0[:])

    # ---- corners: [16384,3,3] -> [128, FCH, 3, 3], face f = p*FCH + ch
    crnT = pkp.tile([128, FCH, 3, 3], F32, tag="pk")
    crn_r = crn[:, :, :].rearrange("(p ch) c d -> p ch c d", p=128)
    for part in range(8):
        nc.sync.dma_start(
            out=crnT[:, part * 16:(part + 1) * 16, :, :],
            in_=crn_r[:, part * 16:(part + 1) * 16, :, :],
        )

    # ---- query side: qT[:, n] = [2qx,2qy,2qz,-1], query (n&127)*16 + (n>>7)
    qRM = sing.tile([128, PC, 3], F32)
    nc.sync.dma_start(out=qRM[:], in_=pp[:, :].rearrange("(p i) c -> p i c", p=128))
    qCM = work.tile([128, 3, PC], F32, tag="qcm")
    nc.vector.tensor_copy(qCM[:], qRM[:].rearrange("p i c -> p c i"))
    qT = sing.tile([4, P], F32)
    nc.vector.memset(qT[:, :], -1.0)
    qtp = mpsum.tile([48, 128], F32, tag="tp")
    nc.tensor.transpose(qtp[:], qCM[:].rearrange("p c i -> p (c i)"), ident[:])
    qtsb = work.tile([48, 128], F32, tag="tsb")
    nc.vector.tensor_scalar(
        out=qtsb[:], in0=qtp[:], scalar1=2.0, scalar2=None, op0=A.mult
    )
    nc.sync.dma_start(
        out=qT[0:3, :].rearrange("c (i p) -> c i p", p=128), in_=qtsb[:]
    )

    # ---- gt side: rT[:, n] = [x,y,z,|r|^2], vertex (n&127)*64 + (n>>7)
    rRM = work.tile([128, 64, 3], F32, tag="rrm")
    nc.sync.dma_start(out=rRM[:], in_=gv[:, :].rearrange("(p t) c -> p t c", p=128))
    rCM = work.tile([128, 2, 3, 32], F32, tag="rcm")
    nc.vector.tensor_copy(rCM[:], rRM[:].rearrange("p (h t) c -> p h c t", h=2))
    sq = work.tile([128, 64, 3], F32, tag="sq")
    nc.vector.tensor_tensor(out=sq[:], in0=rRM[:], in1=rRM[:], op=A.mult)
    rsq = work.tile([128, 64], F32, tag="rsq")
    nc.vector.tensor_reduce(out=rsq[:], in_=sq[:], axis=AX.X, op=A.add)
    rT = sing.tile([4, NGT], F32)
    for h in range(2):
        ctp = mpsum.tile([96, 128], F32, tag="tp")
        nc.tensor.transpose(
            ctp[:], rCM[:, h, :, :].rearrange("p c t -> p (c t)"), ident[:]
        )
        ctsb = work.tile([96, 128], F32, tag="tsb")
        nc.vector.tensor_copy(ctsb[:], ctp[:])
        nc.sync.dma_start(
            out=rT[0:3, h * 32 * 128 : (h + 1) * 32 * 128].rearrange(
                "c (t p) -> c t p", p=128
            ),
            in_=ctsb[:],
        )
    stp = mpsum.tile([64, 128], F32, tag="tp")
    nc.tensor.transpose(stp[:], rsq[:], ident[:])
    stsb = work.tile([64, 128], F32, tag="tsb")
    nc.vector.tensor_copy(stsb[:], stp[:])
    nc.sync.dma_start(out=rT[3:4, :], in_=stsb[:])

    # ---- pred side (padded to 2688): vertex (n&127)*21 + (n>>7)
    rRMp = work.tile([128, 21, 3], F32, tag="rrmp")
    nc.vector.memset(rRMp[:], BIGC)
    rRMp_f = rRMp[:].rearrange("p t c -> p (t c)")
    pv_f = pv[:, :].rearrange("v c -> (v c)")
    nc.sync.dma_start(
        out=rRMp_f[0:122, :],
        in_=pv_f[0 : 122 * 63].rearrange("(p a) -> p a", a=63),
    )
    rCMp = work.tile([128, 3, 21], F32, tag="rcmp")
    nc.vector.tensor_copy(rCMp[:], rRMp[:].rearrange("p t c -> p c t"))
    sqp = work.tile([128, 21, 3], F32, tag="sqp")
    nc.vector.tensor_tensor(out=sqp[:], in0=rRMp[:], in1=rRMp[:], op=A.mult)
    rsqp = work.tile([128, 21], F32, tag="rsqp")
    nc.vector.tensor_reduce(out=rsqp[:], in_=sqp[:], axis=AX.X, op=A.add)
    rTp = sing.tile([4, VPAD], F32)
    ptp = mpsum.tile([63, 128], F32, tag="tp")
    nc.tensor.transpose(ptp[:], rCMp[:].rearrange("p c t -> p (c t)"), ident[:])
    ptsb = work.tile([63, 128], F32, tag="tsb")
    nc.vector.tensor_copy(ptsb[:], ptp[:])
    nc.sync.dma_start(
        out=rTp[0:3, :].rearrange("c (t p) -> c t p", p=128), in_=ptsb[:]
    )
    sptp = mpsum.tile([21, 128], F32, tag="tp")
    nc.tensor.transpose(sptp[:], rsqp[:], ident[:])
    sptsb = work.tile([21, 128], F32, tag="tsb")
    nc.vector.tensor_copy(sptsb[:], sptp[:])
    nc.sync.dma_start(out=rTp[3:4, :], in_=sptsb[:])

    # ---------------- faces: corner indices, lo/hi decomposition ----------
    faces = sing.tile([128, FCH, 3], I32)
    nc.sync.dma_start(
        out=faces[:], in_=gf[:, :].rearrange("(p ch) w -> p ch w", p=128)
    )
    lo_i = sing.tile([128, FCH, 3], I32)
    hi_i = sing.tile([128, FCH, 3], I32)
    nc.vector.tensor_scalar(
        out=lo_i[:], in0=faces[:], scalar1=127, scalar2=None, op0=A.bitwise_and
    )
    nc.vector.tensor_scalar(
        out=hi_i[:], in0=faces[:], scalar1=7, scalar2=None, op0=A.logical_shift_right
    )
    lo_f = sing.tile([128, FCH, 3], F32)
    hi_f = sing.tile([128, FCH, 3], F32)
    nc.vector.tensor_copy(lo_f[:], lo_i[:])
    nc.vector.tensor_copy(hi_f[:], hi_i[:])

    # ---------------- iotas ----------------
    io128_i = sing.tile([128, 128], I32)
    nc.gpsimd.iota(io128_i[:], pattern=[[1, 128]], base=0, channel_multiplier=0)
    io128 = sing.tile([128, 128], F16)
    nc.vector.tensor_copy(io128[:], io128_i[:])
    io64_i = sing.tile([128, 64], I32)
    nc.gpsimd.iota(io64_i[:], pattern=[[1, 64]], base=0, channel_multiplier=0)
    io64 = sing.tile([128, 64], F16)
    nc.vector.tensor_copy(io64[:], io64_i[:])
    iogi = pkp.tile([128, NGT], I32, tag="pk")
    nc.gpsimd.iota(iogi[:], pattern=[[1, NGT]], base=0, channel_multiplier=0)
    iof = sing.tile([128, NGT], F32)
    nc.vector.tensor_scalar(
        out=iof[:], in0=iogi[:], scalar1=IOS, scalar2=None, op0=A.mult
    )

    # ---------------- cross products from host-gathered corners -----------
    eA = sing.tile([128, FCH, 3], F32)
    eB = sing.tile([128, FCH, 3], F32)
    nc.vector.tensor_tensor(
        out=eA[:], in0=crnT[:, :, 1, :], in1=crnT[:, :, 0, :], op=A.subtract
    )
    nc.vector.tensor_tensor(
        out=eB[:], in0=crnT[:, :, 2, :], in1=crnT[:, :, 0, :], op=A.subtract
    )
    fn = sing.tile([128, FCH, 3], F32)
    for d in range(3):
        u, v = (d + 1) % 3, (d + 2) % 3
        t1 = work.tile([128, FCH], F32, tag="cr1")
        t2 = work.tile([128, FCH], F32, tag="cr2")
        nc.vector.tensor_tensor(out=t1[:], in0=eA[:, :, u], in1=eB[:, :, v], op=A.mult)
        nc.vector.tensor_tensor(out=t2[:], in0=eA[:, :, v], in1=eB[:, :, u], op=A.mult)
        nc.vector.tensor_tensor(out=fn[:, :, d], in0=t1[:], in1=t2[:], op=A.subtract)

    # ---------------- one-hot scatter state: G[lo, c, hi] -----------------
    Gp = gpsum.tile([128, 3, 64], F32)
    _oh_state = {"k": 0, "mm": 0, "pend": []}
    OH_LAG = 6

    def _emit_gmm():
        km = _oh_state["mm"]
        ohlo, R = _oh_state["pend"].pop(0)
        nc.tensor.matmul(
            Gp[:],
            ohlo[:],
            R[:],
            start=(km == 0),
            stop=(km == 3 * FCH - 1),
            skip_group_check=True,
        )
        _oh_state["mm"] = km + 1

    def emit_onehot(n):
        for _ in range(n):
            k = _oh_state["k"]
            if k >= 3 * FCH:
                while _oh_state["pend"]:
                    _emit_gmm()
                return
            ch, corner = divmod(k, 3)
            ohlo = oh.tile([128, 128], F16, tag="ohlo")
            nc.vector.tensor_scalar(
                out=ohlo[:],
                in0=io128[:],
                scalar1=lo_f[:, ch : ch + 1, corner : corner + 1],
                scalar2=None,
                op0=A.is_equal,
            )
            R = oh.tile([128, 3, 64], F16, tag="R")
            for d in range(3):
                nc.gpsimd.tensor_scalar(
                    out=R[:, d, :],
                    in0=io64[:],
                    scalar1=hi_f[:, ch : ch + 1, corner : corner + 1],
                    scalar2=fn[:, ch : ch + 1, d : d + 1],
                    op0=A.is_equal,
                    op1=A.mult,
                )
            _oh_state["pend"].append((ohlo, R))
            if len(_oh_state["pend"]) > OH_LAG:
                _emit_gmm()
            _oh_state["k"] = k + 1

    # ---------------- search machinery ----------------
    pkmax_gt = sing.tile([128, PC], F32)
    pkmax_pr = sing.tile([128, PC], F32)

    def search_qchunk(i, rT_t, ncols, groups, evrow, pkmax_out):
        qTi = qT[:, i * 128 : (i + 1) * 128].bitcast(F32R)
        samp = work.tile([128, 1], F32, tag="samp")
        bias = work.tile([128, 1], F32, tag="bias")
        first = True
        for g0, g1 in groups:
            w = g1 - g0
            ps = spsum.tile([128, 1536], F32, tag="s")
            for c0 in range(0, w, 512):
                cw = min(512, w - c0)
                nc.tensor.matmul(
                    ps[:, c0 : c0 + cw],
                    qTi,
                    rT_t[:, g0 + c0 : g0 + c0 + cw].bitcast(F32R),
                    start=True,
                    stop=True,
                )
            if first:
                # row-max estimate from a strided sample of group 0
                sj = work.tile([128, w // 8], F32, tag="sj")
                nc.gpsimd.tensor_scalar(
                    out=sj[:], in0=ps[:, 0:w:8], scalar1=NEG, scalar2=None,
                    op0=A.max, op1=A.max, accum_out=samp[:],
                )
                # bias = 1024 + (1 - m^)*256
                nc.vector.tensor_scalar(
                    out=bias[:], in0=samp[:], scalar1=-WSC, scalar2=1024.0 + WSC,
                    op0=A.mult, op1=A.add,
                )
                first = False
            nc.scalar.activation(
                evrow[:, g0:g1], ps[:, 0:w], AF.Relu, bias=bias[:], scale=WSC
            )
        # pk pass split across DVE / GPSIMD (tensor_tensor is 1x on both;
        # Pool is 1.33x slower so it gets the smaller share)
        if ncols == NGT:
            wA = 4096
            pk = pkp.tile([128, NGT], F32, tag="pk")
        else:
            wA = 1344
            pk = pkp.tile([128, VPAD], F32, tag="pkpr")
        nc.vector.tensor_tensor(
            out=pk[:, 0:wA], in0=evrow[:, 0:wA], in1=iof[:, 0:wA], op=A.add
        )
        nc.gpsimd.tensor_tensor(
            out=pk[:, wA:ncols], in0=evrow[:, wA:ncols], in1=iof[:, wA:ncols],
            op=A.add,
        )
        nc.vector.tensor_scalar(
            out=pk[:, 0:ncols], in0=pk[:, 0:ncols], scalar1=NEG, scalar2=None,
            op0=A.max, op1=A.max, accum_out=pkmax_out[:, i : i + 1],
        )

    # gt-group chunk boundaries in columns
    gtg = [(a * 512, b * 512) for a, b in GT_GROUPS]

    # ---------------- main interleaved loop ----------------
    for i in range(PC):
        evg = evp.tile([128, NGT], F16, tag="evg")
        search_qchunk(i, rT, NGT, gtg, evg, pkmax_gt)
        emit_onehot(12)
        evp_p = evp.tile([128, VPAD], F16, tag="evp")
        search_qchunk(i, rTp, VPAD, PR_GROUPS, evp_p, pkmax_pr)
        emit_onehot(12)
        if dbg and i == 0:
            nc.sync.dma_start(out=dbg["dbg_evg"][:, :], in_=evg[:])
    emit_onehot(3 * FCH)  # leftovers

    # ---------------- decode pk -> column j -> vertex id ----------------
    def decode(pkmax_t, idx_t, mult):
        jf = work.tile([128, PC], F32, tag="jf")
        nc.vector.tensor_scalar(
            out=jf[:], in0=pkmax_t[:], scalar1=8192.0, scalar2=None, op0=A.mult
        )
        ji = work.tile([128, PC], I32, tag="ji")
        nc.vector.tensor_copy(ji[:], jf[:])
        nc.vector.tensor_scalar(
            out=ji[:], in0=ji[:], scalar1=8191, scalar2=None, op0=A.bitwise_and
        )
        # vertex = (j&127)*mult + (j>>7)
        a = work.tile([128, PC], I32, tag="ua")
        bcol = work.tile([128, PC], I32, tag="ub")
        nc.vector.tensor_scalar(
            out=a[:], in0=ji[:], scalar1=127, scalar2=mult, op0=A.bitwise_and,
            op1=A.mult,
        )
        nc.vector.tensor_scalar(
            out=bcol[:], in0=ji[:], scalar1=7, scalar2=None,
            op0=A.logical_shift_right,
        )
        nc.vector.tensor_tensor(out=idx_t[:], in0=a[:], in1=bcol[:], op=A.add)

    idx_gt = sing.tile([128, PC], I32)
    idx_pr = sing.tile([128, PC], I32)
    decode(pkmax_gt, idx_gt, 64)
    decode(pkmax_pr, idx_pr, 21)

    # ---------------- G -> DRAM (queue-split) ----------------
    Gs = sing.tile([128, 3, 64], F32)
    nc.scalar.copy(Gs[:], Gp[:])
    Gs2 = sing.tile([128, 64, 3], F32)
    nc.vector.tensor_copy(Gs2[:], Gs[:].rearrange("p c h -> p h c"))
    g_r = g_dram[:, :].rearrange(
        "(lo hi c) one -> lo (hi c one)", lo=128, hi=64
    )
    for part in range(8):
        nc.sync.dma_start(
            out=g_r[:, part * 24:(part + 1) * 24],
            in_=Gs2[:].rearrange("p h c -> p (h c)")[:, part * 24:(part + 1) * 24],
        )

    # gather offsets for normals: (v & 127)*192 + (v >> 7)*3
    o1 = work.tile([128, PC], I32, tag="o1")
    o2 = work.tile([128, PC], I32, tag="o2")
    nc.vector.tensor_scalar(
        out=o1[:], in0=idx_gt[:], scalar1=127, scalar2=192, op0=A.bitwise_and,
        op1=A.mult,
    )
    nc.vector.tensor_scalar(
        out=o2[:], in0=idx_gt[:], scalar1=7, scalar2=3,
        op0=A.logical_shift_right, op1=A.mult,
    )
    offs = sing.tile([128, PC], I32)
    nc.vector.tensor_tensor(out=offs[:], in0=o1[:], in1=o2[:], op=A.add)

    nGT = sing.tile([128, PC, 3], F32)
    for part in range(4):
        nc.gpsimd.indirect_dma_start(
            out=nGT[:, part * 4:(part + 1) * 4, :],
            out_offset=None,
            in_=g_dram[:, :],
            in_offset=bass.IndirectOffsetOnAxis(
                ap=offs[:, part * 4:(part + 1) * 4], axis=0
            ),
        )

    # nearest pred vertices
    idx_pr3 = sing.tile([128, PC], I32)
    nc.vector.tensor_scalar(
        out=idx_pr3[:], in0=idx_pr[:], scalar1=3, scalar2=None, op0=A.mult
    )
    pv_flat2 = pv[:, :].rearrange("v (c one) -> (v c) one", one=1)
    vNN = sing.tile([128, PC, 3], F32)
    for part in range(4):
        nc.gpsimd.indirect_dma_start(
            out=vNN[:, part * 4:(part + 1) * 4, :],
            out_offset=None,
            in_=pv_flat2,
            in_offset=bass.IndirectOffsetOnAxis(
                ap=idx_pr3[:, part * 4:(part + 1) * 4], axis=0
            ),
        )

    # ---------------- epilogue ----------------
    e = sing.tile([128, PC, 3], F32)
    nc.vector.tensor_tensor(out=e[:], in0=qRM[:], in1=vNN[:], op=A.subtract)
    tmp3 = work.tile([128, PC, 3], F32, tag="en")
    nc.vector.tensor_tensor(out=tmp3[:], in0=e[:], in1=nGT[:], op=A.mult)
    dot = sing.tile([128, PC], F32)
    nc.vector.tensor_reduce(out=dot[:], in_=tmp3[:], axis=AX.X, op=A.add)
    ee_t = work.tile([128, PC, 3], F32, tag="en")
    nc.vector.tensor_tensor(out=ee_t[:], in0=e[:], in1=e[:], op=A.mult)
    ee = sing.tile([128, PC], F32)
    nc.vector.tensor_reduce(out=ee[:], in_=ee_t[:], axis=AX.X, op=A.add)
    nn_t = work.tile([128, PC, 3], F32, tag="en")
    nc.vector.tensor_tensor(out=nn_t[:], in0=nGT[:], in1=nGT[:], op=A.mult)
    nn = sing.tile([128, PC], F32)
    nc.vector.tensor_reduce(out=nn[:], in_=nn_t[:], axis=AX.X, op=A.add)

    elen = sing.tile([128, PC], F32)
    nlen = sing.tile([128, PC], F32)
    nc.scalar.activation(elen[:], ee[:], AF.Sqrt)
    nc.scalar.activation(nlen[:], nn[:], AF.Sqrt)
    nc.vector.tensor_scalar(
        out=elen[:], in0=elen[:], scalar1=EPS, scalar2=None, op0=A.max
    )
    nc.vector.tensor_scalar(
        out=nlen[:], in0=nlen[:], scalar1=EPS, scalar2=None, op0=A.max
    )
    den = sing.tile([128, PC], F32)
    nc.vector.tensor_tensor(out=den[:], in0=elen[:], in1=nlen[:], op=A.mult)
    rden = sing.tile([128, PC], F32)
    nc.vector.reciprocal(rden[:], den[:])
    res = sing.tile([128, PC], F32)
    nc.vector.tensor_tensor(out=res[:], in0=dot[:], in1=rden[:], op=A.mult)
    partial = sing.tile([128, 1], F32)
    nc.vector.tensor_reduce(
        out=partial[:], in_=res[:], axis=AX.X, op=A.add, apply_absolute_value=True
    )
    ones = sing.tile([128, 1], F32)
    nc.vector.memset(ones[:], 1.0)
    fps = mpsum.tile([1, 1], F32, tag="tp")
    nc.tensor.matmul(fps[:], ones[:], partial[:], start=True, stop=True)
    osb = sing.tile([1, 1], F32)
    nc.scalar.copy(osb[:], fps[:])
    nc.sync.dma_start(out=out_dram[:], in_=osb[:])
    if dbg:
        nc.sync.dma_start(out=dbg["dbg_idx_gt"][:, :], in_=idx_gt[:])
        nc.sync.dma_start(out=dbg["dbg_idx_pr"][:, :], in_=idx_pr[:])
        nc.sync.dma_start(out=dbg["dbg_pkg"][:, :], in_=pkmax_gt[:])
        nc.sync.dma_start(out=dbg["dbg_pkp"][:, :], in_=pkmax_pr[:])
        nc.sync.dma_start(out=dbg["dbg_res"][:, :], in_=res[:])
        nc.sync.dma_start(out=dbg["dbg_g"][:, :], in_=Gs2[:].rearrange("p a b -> p (a b)"))
        nc.sync.dma_start(out=dbg["dbg_fn"][:, :], in_=fn[:].rearrange("p a b -> p (a b)"))


_NC_CACHE = None


def _get_nc():
    global _NC_CACHE
    if _NC_CACHE is None:
        _NC_CACHE = build_nc()
    return _NC_CACHE


def make_in_maps(pred_points, pred_vertices, gt_vertices, gt_faces):
    nb = pred_points.shape[0]
    faces32 = np.asarray(gt_faces).astype(np.int32, copy=False)
    out = []
    for b in range(nb):
        gvb = np.ascontiguousarray(gt_vertices[b], dtype=np.float32)
        fb = np.ascontiguousarray(faces32[b])
        out.append(
            dict(
                pred_points=np.ascontiguousarray(pred_points[b], dtype=np.float32),
                pred_vertices=np.ascontiguousarray(pred_vertices[b], dtype=np.float32),
                gt_vertices=gvb,
                gt_faces32=fb,
                corners=np.ascontiguousarray(gvb[fb]),  # [NF, 3, 3]
            )
        )
    return out


def kernel(pred_points, pred_vertices, gt_vertices, gt_faces):
    from concourse.bass_utils import run_bass_kernel_spmd

    nb = pred_points.shape[0]
    nc = _get_nc()
    in_maps = make_in_maps(pred_points, pred_vertices, gt_vertices, gt_faces)
    res = run_bass_kernel_spmd(nc, in_maps, list(range(nb)))
    total = sum(float(res.results[i]["loss_sum"][0]) for i in range(nb))
    return np.array(total / (nb * P), dtype=np.float32)


if __name__ == "__main__":
    nc = build_nc()
    print("built ok")
